# revision 8
# baseline (speedup 1.0000x reference)
"""TRN2 Bass kernel for fused MHA (softmax-over-query quirk) + out-proj + residual + LayerNorm.

Problem shapes (hardcoded): tokens [4,2048,1024], Wq/Wk [16,1024,64], Wv [16,1024,64],
Wo [1024,1024], gamma/beta [1024]. Output [4,2048,1024] fp32.

Sharding: 8 cores, core c owns (batch b=c//2, S-half jc=c%2) of the OUTPUT rows.
No collectives. Each core computes, for its batch b:
  qT[dk,i] (full S), kT[dk,j] (its half), V[i,dv] (full S) in bf16,
  scores^T[i,j] = q_i.k_j (PSUM fp32), e = exp(scores/8) (bf16),
  heads^T[dv,j] + rowsum row via a ones-column appended to V,
  multi^T = heads^T / rowsum, out = multi @ Wo + tokens, LayerNorm rows.

QKV projections run in fp8e4m3 with DoubleRow perf mode (2 K-planes per
matmul): tokens cast to fp8, weights scaled x256 (dodges e4m3 subnormals)
and cast to fp8. The x256 scale cancels: scores pick up 2^16 (folded into
the exp scale) and heads/rowsum both pick up 2^8 (ones column = 256).
Projections are interleaved into the attention stream as hooks so the
Scalar-engine exp stream starts ~10us in instead of after all projections.
Attention math in bf16 matmuls with fp32 PSUM; residual + LN in fp32.
"""

import numpy as np
import ml_dtypes

BF16 = ml_dtypes.bfloat16
FP8 = ml_dtypes.float8_e4m3

B, S, D, H, DK, DV = 4, 2048, 1024, 16, 64, 64
NCORES = 8
NPAIR = 8     # head pairs
NKC = 8       # D // 128 contraction chunks
NIC = 16      # S // 128 i-chunks
JW = 1024     # j columns per core (S/2)
NJCH = 8      # JW // 128
LN_EPS = 1e-5
WSCALE = 256.0  # fp8 weight pre-scale (power of 2)

_CACHE = {}


def _build_nc(apply_affine):
    import concourse.tile as tile
    from concourse import bacc, mybir

    F32 = mybir.dt.float32
    BF = mybir.dt.bfloat16
    F8 = mybir.dt.float8e4
    Exp = mybir.ActivationFunctionType.Exp
    Square = mybir.ActivationFunctionType.Square
    Sqrt = mybir.ActivationFunctionType.Sqrt
    mult = mybir.AluOpType.mult
    add = mybir.AluOpType.add
    DR = mybir.MatmulPerfMode.DoubleRow

    nc = bacc.Bacc(
        "TRN2",
        target_bir_lowering=False,
        debug=False,
        enable_asserts=False,
        num_devices=NCORES,
    )

    # DRAM I/O (per-core views; host prepares layouts)
    tokT_d = nc.dram_tensor("tokT", (128, NKC, S), F8, kind="ExternalInput").ap()
    tokTj_d = nc.dram_tensor("tokTj", (128, NKC, JW), F8, kind="ExternalInput").ap()
    wq_d = nc.dram_tensor("wq", (128, NKC, H * DK), F8, kind="ExternalInput").ap()
    wk_d = nc.dram_tensor("wk", (128, NKC, H * DK), F8, kind="ExternalInput").ap()
    wv_d = nc.dram_tensor("wv", (128, NKC, H * DV), F8, kind="ExternalInput").ap()
    wo_d = nc.dram_tensor("wo", (128, NKC, D), BF, kind="ExternalInput").ap()
    tokres_d = nc.dram_tensor("tokres", (128, NJCH, D), F32, kind="ExternalInput").ap()
    if apply_affine:
        gamma_d = nc.dram_tensor("gamma_bc", (128, D), F32, kind="ExternalInput").ap()
        beta_d = nc.dram_tensor("beta_bc", (128, D), F32, kind="ExternalInput").ap()
    out_d = nc.dram_tensor("out", (128, NJCH, D), F32, kind="ExternalOutput").ap()
    from contextlib import ExitStack

    from concourse.bass import _add_dep_helper

    # Chain all PE matmuls in emission order: stops the scheduler from
    # interleaving row-conflicting matmuls and keeps the stream dense.
    _prev_mm = [None]

    def mm(*args, **kwargs):
        inst = nc.tensor.matmul(*args, **kwargs)
        if _prev_mm[0] is not None:
            _add_dep_helper(inst.ins, _prev_mm[0].ins, sync=False, reason="pe-order")
        _prev_mm[0] = inst
        return inst

    with tile.TileContext(nc) as tc, ExitStack() as stack:
        persist = stack.enter_context(tc.tile_pool(name="persist", bufs=1))
        qT_sb = persist.tile([128, NPAIR, S], BF)          # [pair-dk, pr, i]
        kT_sb = persist.tile([128, NPAIR, JW], BF)         # [pair-dk, pr, j]
        v_sb = persist.tile([128, NIC, H, DV + 1], BF)     # [i%128, ic, h, dv|256s]
        # multi^T stored as one tile per 128-row chunk so out-proj dep tracking
        # stays per-pair (a single big tile serializes on the last DMA write)
        multiT = [
            persist.tile([128, JW], BF, name=f"multiT{kc}") for kc in range(NKC)
        ]
        eps_sb = persist.tile([128, 1], F32)
        if apply_affine:
            gamma_sb = persist.tile([128, D], F32)
            beta_sb = persist.tile([128, D], F32)
            nc.sync.dma_start(gamma_sb[:], gamma_d[:])
            nc.sync.dma_start(beta_sb[:], beta_d[:])
        nc.vector.memset(eps_sb[:], LN_EPS)
        for ic in range(NIC):
            # ones column scaled by WSCALE so rowsum matches the x256 V scale
            nc.vector.memset(v_sb[:, ic, :, DV : DV + 1], WSCALE)

        # pools that outlive pa must be allocated first (LIFO release)
        psS = tc.alloc_tile_pool(name="psS", bufs=2, space="PSUM")
        psAcc = tc.alloc_tile_pool(name="psAcc", bufs=2, space="PSUM")
        pe_pool = stack.enter_context(tc.tile_pool(name="pe", bufs=6))
        pn_pool = stack.enter_context(tc.tile_pool(name="pn", bufs=2))
        pdram = stack.enter_context(tc.tile_pool(name="pdram", bufs=2, space="DRAM"))

        pa = tc.alloc_tile_pool(name="pa", bufs=1)
        wq_sb = pa.tile([128, NKC, H * DK], F8)
        wk_sb = pa.tile([128, NKC, H * DK], F8)
        tokT_sb = pa.tile([128, NKC, S], F8)
        tokTj_sb = pa.tile([128, NKC, JW], F8)
        wv_sb = pa.tile([128, NKC, H * DV], F8)

        # Startup DMA across all three issue paths (SP + ACT HWDGE rings run
        # FIFO independently; gpsimd is SWDGE) so the critical ~3.5MB for the
        # first scores lands in ~1/3 the serial time.
        nc.sync.dma_start(wq_sb[:], wq_d[:])
        for kc in range(NKC):  # tokens i 0..511 first (q chain t0, V chains)
            nc.scalar.dma_start(tokT_sb[:, kc, 0:512], tokT_d[:, kc, 0:512])
        nc.sync.dma_start(wk_sb[:], wk_d[:])
        for kc in range(NKC):
            nc.scalar.dma_start(tokTj_sb[:, kc], tokTj_d[:, kc])
        for kc in range(NKC):
            nc.gpsimd.dma_start(wv_sb[:, kc], wv_d[:, kc])
        for t in range(1, 4):
            for kc in range(NKC):
                nc.gpsimd.dma_start(
                    tokT_sb[:, kc, t * 512 : (t + 1) * 512],
                    tokT_d[:, kc, t * 512 : (t + 1) * 512],
                )

        def proj_chain(pr, which, t):
            """One 512-wide fp8 DoubleRow projection chain via a borrowed
            scores-pool slot."""
            w_sb, dst, rhs_sb = (
                (wq_sb, qT_sb, tokT_sb) if which == "q" else (wk_sb, kT_sb, tokTj_sb)
            )
            ps = psS.tile([128, 512], F32, tag="sc", name=f"pj{which}{pr}_{t}")
            for kc in range(0, NKC, 2):
                mm(
                    ps[:],
                    w_sb[:, kc : kc + 2, pr * 128 : (pr + 1) * 128],
                    rhs_sb[:, kc : kc + 2, t * 512 : (t + 1) * 512],
                    start=(kc == 0),
                    stop=(kc == NKC - 2),
                    perf_mode=DR,
                )
            nc.vector.tensor_copy(out=dst[:, pr, t * 512 : (t + 1) * 512], in_=ps[:])

        def proj_v(ic):
            """fp8 DoubleRow V projection for one i-chunk via a borrowed
            scores-pool slot."""
            ps = psS.tile([128, 1024], F32, tag="sc", name=f"pjv{ic}")
            for kc in range(0, NKC, 2):
                for nb in range(2):
                    mm(
                        ps[:, nb * 512 : (nb + 1) * 512],
                        tokT_sb[:, kc : kc + 2, ic * 128 : (ic + 1) * 128],
                        wv_sb[:, kc : kc + 2, nb * 512 : (nb + 1) * 512],
                        start=(kc == 0),
                        stop=(kc == NKC - 2),
                        perf_mode=DR,
                    )
            nc.vector.tensor_copy(
                out=v_sb[:, ic, :, 0:DV], in_=ps.rearrange("p (h v) -> p h v", h=H)
            )

        def normalize(pr, acc):
            """multi^T[h] = heads^T / rowsum; runs on DVE/DMA only."""
            for hh in range(2):
                h = 2 * pr + hh
                hraw = pn_pool.tile([DV + 1, JW], F32, tag="hraw", name=f"hraw{h}")
                nc.vector.tensor_copy(out=hraw[:], in_=acc[hh][:])  # frees acc
                rs_dram = pdram.tile([1, JW], F32, tag="rsd", name=f"rsd{h}")
                nc.sync.dma_start(out=rs_dram[:], in_=hraw[DV : DV + 1, :])
                rec_in = pn_pool.tile([DV, JW], F32, tag="rin", name=f"rin{h}")
                nc.gpsimd.dma_start(out=rec_in[:], in_=rs_dram.to_broadcast((DV, JW)))
                nc.vector.reciprocal_approx_fast(out=rec_in[:], in_=rec_in[:])
                if hh == 0:
                    nc.vector.tensor_tensor(
                        multiT[h // 2][0:64, :], hraw[0:DV, :], rec_in[:], mult
                    )
                else:
                    tmp64 = pn_pool.tile([DV, JW], BF, tag="tmp64", name=f"tmp{h}")
                    nc.vector.tensor_tensor(tmp64[:], hraw[0:DV, :], rec_in[:], mult)
                    nc.sync.dma_start(out=multiT[h // 2][64:128, :], in_=tmp64[:])

        def attention(hooks_by_pr, after_pair=None):
            """All pairs, flat: attnV lags one i-chunk behind scores/exp and
            crosses pair boundaries so the exp stream never waits on the PE.
            Hooks and the lagged attnV are emitted BEFORE this iteration's
            scores: the scores matmul must wait for an exp to free its PSUM
            slot (ring depth 2), and everything emitted ahead of it in the PE
            chain fills that wait.  hooks_by_pr[pr][ic] is a list of thunks."""
            pending_av = None   # (eTs, ic, pr, acc)
            acc_by_pr = {}

            def do_attnv(peT, pic, ppr, pacc):
                for hh in range(2):
                    for jb in range(2):
                        mm(
                            pacc[hh][:, jb * 512 : (jb + 1) * 512],
                            v_sb[:, pic, 2 * ppr + hh, :],
                            peT[hh][:, jb * 512 : (jb + 1) * 512],
                            start=(pic == 0),
                            stop=(pic == NIC - 1),
                        )

            for pr in range(NPAIR):
                acc_by_pr[pr] = [
                    psAcc.tile([DV + 1, JW], F32, tag="acc", name=f"acc{pr}_{hh}")
                    for hh in range(2)
                ]
                hooks = hooks_by_pr.get(pr, {})
                for ic in range(NIC):
                    for fn in hooks.get(ic, ()):
                        fn()
                    if pending_av is not None:
                        do_attnv(*pending_av)
                        if pending_av[1] == NIC - 1:
                            normalize(pending_av[2], pending_av[3])
                    ps_s = [
                        psS.tile([128, JW], F32, tag="sc", name=f"ps_s{pr}_{ic}_{hh}")
                        for hh in range(2)
                    ]
                    # scores^T, row-tiled pair (K=64 at partitions 0/64)
                    for hh in range(2):
                        for jb in range(2):
                            mm(
                                ps_s[hh][:, jb * 512 : (jb + 1) * 512],
                                qT_sb[hh * 64 : (hh + 1) * 64, pr, ic * 128 : (ic + 1) * 128],
                                kT_sb[hh * 64 : (hh + 1) * 64, pr, jb * 512 : (jb + 1) * 512],
                                start=True,
                                stop=True,
                            )
                    eTs = []
                    for hh in range(2):
                        eT = pe_pool.tile([128, JW], BF, tag="eT", name=f"eT{pr}_{ic}_{hh}")
                        # x256-scaled q and k: fold 2^-16 into the exp scale
                        nc.scalar.activation(
                            eT[:], ps_s[hh][:], Exp, scale=0.125 / (WSCALE * WSCALE)
                        )
                        eTs.append(eT)
                    pending_av = (eTs, ic, pr, acc_by_pr[pr])
                if after_pair and pr in after_pair:
                    after_pair[pr]()
            do_attnv(*pending_av)
            normalize(pending_av[2], pending_av[3])

        # Hook schedule: V projections ride pair 0; each pair p computes its
        # own q chains t1-3 mid-pair and pair p+1's q t0 / k chains late, so
        # every pair's inputs are ready one pair ahead. Pair 7's t1-3 move
        # into pair 6 so tokT dies at pair 6's end (phase C reuses the space).
        hooks = {pr: {} for pr in range(NPAIR)}

        def add_hook(pr, ic, fn):
            hooks[pr].setdefault(ic, []).append(fn)

        for ic in range(2, NIC):
            add_hook(0, ic, lambda ic=ic: proj_v(ic))
        add_hook(0, 1, lambda: proj_v(0))
        add_hook(0, 1, lambda: proj_v(1))
        for pr in range(NPAIR):
            own = pr if pr < 7 else 6
            for t, ic in (
                ((1, 3), (2, 7), (3, 11)) if pr < 7 else ((1, 9), (2, 10), (3, 12))
            ):
                add_hook(own, ic, lambda pr=pr, t=t: proj_chain(pr, "q", t))
            if pr < 7:
                add_hook(pr, 13, lambda pr=pr: proj_chain(pr + 1, "q", 0))
                add_hook(pr, 14, lambda pr=pr: proj_chain(pr + 1, "k", 0))
                add_hook(pr, 15, lambda pr=pr: proj_chain(pr + 1, "k", 1))

        pc_tiles = {}

        def open_phase_c():
            # pa's tensors are dead after pair 6 (pair 7's chains were hoisted
            # into pair 6); reuse the space for phase C inputs so their DMA
            # overlaps pair 7.
            pa.release()
            pc = stack.enter_context(tc.tile_pool(name="pc", bufs=1))
            pc_tiles["wo"] = pc.tile([128, NKC, D], BF, name="wo_sb")
            pc_tiles["tokres"] = pc.tile([128, NJCH, D], F32, name="tokres_sb")
            nc.sync.dma_start(pc_tiles["wo"][:], wo_d[:])
            nc.sync.dma_start(pc_tiles["tokres"][:], tokres_d[:])

        # upfront: just enough projection for pair 0's first scores
        proj_chain(0, "q", 0)
        proj_chain(0, "k", 0)
        proj_chain(0, "k", 1)

        attention(hooks, after_pair={6: open_phase_c})
        wo_sb = pc_tiles["wo"]
        tokres_sb = pc_tiles["tokres"]
        psAcc.release()
        psS.release()
        # ---------------- Phase C: out-proj + residual + LayerNorm ----------------
        with (
            tc.tile_pool(name="pC", bufs=2) as pC,
            tc.tile_pool(name="pStats", bufs=8) as pStats,
            tc.tile_pool(name="psC", bufs=4, space="PSUM") as psC,
        ):
            # Out-proj in two steps per jch: kc 0-6 accumulate early (their
            # multiT chunks are ready pairs before the last normalize), kc 7
            # finishes when multiT[7] lands. Prefilling 4 PSUM groups hides
            # the last normalize's DRAM round-trip behind ~12us of matmuls.
            prefill = {}

            def emit_prefill(jch):
                ps_o = psC.tile([128, D], F32, tag="po", name=f"ps_o{jch}")
                for kc in range(NKC - 1):
                    lhsT = multiT[kc][:, jch * 128 : (jch + 1) * 128]
                    for nb in range(2):
                        mm(
                            ps_o[:, nb * 512 : (nb + 1) * 512],
                            lhsT,
                            wo_sb[:, kc, nb * 512 : (nb + 1) * 512],
                            start=(kc == 0),
                            stop=False,
                        )
                prefill[jch] = ps_o

            for jch in range(4):
                emit_prefill(jch)
            for jch in range(NJCH):
                ps_o = prefill.pop(jch)
                lhsT = multiT[NKC - 1][:, jch * 128 : (jch + 1) * 128]
                for nb in range(2):
                    mm(
                        ps_o[:, nb * 512 : (nb + 1) * 512],
                        lhsT,
                        wo_sb[:, NKC - 1, nb * 512 : (nb + 1) * 512],
                        start=False,
                        stop=True,
                    )
                # x = psum + residual, sum_t = rowsum(x), in one DVE pass
                x_sb = pC.tile([128, D], F32, tag="x", name=f"x{jch}")
                sum_t = pStats.tile([128, 1], F32, tag="sum", name=f"sum{jch}")
                nc.vector.scalar_tensor_tensor(
                    out=x_sb[:],
                    in0=ps_o[:],
                    scalar=0.0,
                    in1=tokres_sb[:, jch, :],
                    op0=add,
                    op1=add,
                    accum_out=sum_t[:],
                )
                negmean = pStats.tile([128, 1], F32, tag="nm", name=f"nm{jch}")
                nc.vector.tensor_scalar_mul(negmean[:], sum_t[:], -1.0 / D)
                # ssq = sum((x-m)^2) on the otherwise-idle Scalar engine:
                # Square(x*1 + negmean) with accum_out
                scrap = pC.tile([128, D], BF, tag="scrap", name=f"scrap{jch}")
                ssq = pStats.tile([128, 1], F32, tag="ssq", name=f"ssq{jch}")
                nc.scalar.activation(
                    scrap[:], x_sb[:], Square, bias=negmean[:], accum_out=ssq[:]
                )
                std_t = pStats.tile([128, 1], F32, tag="std", name=f"std{jch}")
                nc.scalar.activation(std_t[:], ssq[:], Sqrt, bias=eps_sb[:], scale=1.0 / D)
                rstd = pStats.tile([128, 1], F32, tag="rstd", name=f"rstd{jch}")
                nc.vector.reciprocal(rstd[:], std_t[:])
                # (x - m) * rstd == x*rstd + (negmean*rstd), one ACT op
                rstd_nm = pStats.tile([128, 1], F32, tag="rnm", name=f"rnm{jch}")
                nc.vector.tensor_tensor(rstd_nm[:], negmean[:], rstd[:], mult)
                out_sb = pC.tile([128, D], F32, tag="out", name=f"out{jch}")
                nc.scalar.activation(
                    out_sb[:],
                    x_sb[:],
                    mybir.ActivationFunctionType.Identity,
                    bias=rstd_nm[:],
                    scale=rstd[:],
                )
                if apply_affine:
                    nc.gpsimd.tensor_tensor(out_sb[:], out_sb[:], gamma_sb[:], mult)
                    nc.gpsimd.tensor_tensor(out_sb[:], out_sb[:], beta_sb[:], add)
                nc.sync.dma_start(out_d[:, jch], out_sb[:])
                # next group's early accumulation (after the STT above so the
                # pool's anti-dependency on ps_o[jch] is complete)
                if jch + 4 < NJCH:
                    emit_prefill(jch + 4)

    nc.compile()
    return nc


def _prep_inputs(tokens, Wq, Wk, Wv, Wo, gamma, beta):
    """Host-side layout prep. Returns per-core input maps."""
    tokens = np.ascontiguousarray(np.asarray(tokens, dtype=np.float32))
    # weights -> [p, kc, n] with row index kc*128+p
    def rows128(a):  # [1024, N] -> [128, 8, N]
        return np.ascontiguousarray(
            a.reshape(NKC, 128, a.shape[-1]).transpose(1, 0, 2)
        )

    wq_all = rows128(
        (np.asarray(Wq).transpose(1, 0, 2).reshape(D, H * DK) * WSCALE).astype(FP8)
    )
    wk_all = rows128(
        (np.asarray(Wk).transpose(1, 0, 2).reshape(D, H * DK) * WSCALE).astype(FP8)
    )
    wv_all = rows128(
        (np.asarray(Wv).transpose(1, 0, 2).reshape(D, H * DV) * WSCALE).astype(FP8)
    )
    wo_all = rows128(np.asarray(Wo).astype(BF16))
    gamma_bc = np.ascontiguousarray(
        np.broadcast_to(np.asarray(gamma, np.float32), (128, D))
    )
    beta_bc = np.ascontiguousarray(
        np.broadcast_to(np.asarray(beta, np.float32), (128, D))
    )

    tokT_by_b = []
    for b in range(B):
        tokT_by_b.append(rows128(tokens[b].T.astype(FP8)))  # [128, 8, 2048]

    in_maps = []
    for c in range(NCORES):
        b, jc = c // 2, c % 2
        tokT = tokT_by_b[b]
        tokTj = np.ascontiguousarray(tokT[:, :, jc * JW : (jc + 1) * JW])
        tokres = np.ascontiguousarray(
            tokens[b, jc * JW : (jc + 1) * JW]
            .reshape(NJCH, 128, D)
            .transpose(1, 0, 2)
        )
        in_maps.append(
            {
                "tokT": tokT,
                "tokTj": tokTj,
                "wq": wq_all,
                "wk": wk_all,
                "wv": wv_all,
                "wo": wo_all,
                "tokres": tokres,
                "gamma_bc": gamma_bc,
                "beta_bc": beta_bc,
            }
        )
    return in_maps


def run(inputs, trace=False, tmpdir=None):
    """Run on hardware; returns (output, BassKernelResults)."""
    from concourse.bass_utils import run_bass_kernel_spmd

    apply_affine = not (
        np.all(np.asarray(inputs["gamma"]) == 1.0)
        and np.all(np.asarray(inputs["beta"]) == 0.0)
    )
    key = ("nc", apply_affine)
    if key not in _CACHE:
        _CACHE[key] = _build_nc(apply_affine)
    nc = _CACHE[key]
    in_maps = _prep_inputs(**inputs)
    res = run_bass_kernel_spmd(
        nc, in_maps, core_ids=list(range(NCORES)), trace=trace, tmpdir=tmpdir
    )
    out = np.empty((B, S, D), np.float32)
    for c in range(NCORES):
        b, jc = c // 2, c % 2
        o = res.results[c]["out"]  # [128, 8, 1024]
        out[b, jc * JW : (jc + 1) * JW] = (
            o.transpose(1, 0, 2).reshape(JW, D)
        )
    return out, res


def kernel(tokens, Wq, Wk, Wv, Wo, gamma, beta):
    out, _ = run(
        dict(tokens=tokens, Wq=Wq, Wk=Wk, Wv=Wv, Wo=Wo, gamma=gamma, beta=beta)
    )
    return out


# revision 19
# speedup vs baseline: 1.2599x; 1.2599x over previous
"""TRN2 Bass kernel for fused MHA (softmax-over-query quirk) + out-proj + residual + LayerNorm.

Problem shapes (hardcoded): tokens [4,2048,1024], Wq/Wk [16,1024,64], Wv [16,1024,64],
Wo [1024,1024], gamma/beta [1024]. Output [4,2048,1024] fp32.

Sharding: 8 cores, core c owns (batch b=c//2, S-half jc=c%2) of the OUTPUT rows.
No collectives. Each core computes, for its batch b:
  qT[dk,i] (full S), kT[dk,j] (its half), V[i,dv] (full S) in bf16,
  scores^T[i,j] = q_i.k_j (PSUM fp32), e = exp(scores/8) (bf16),
  heads^T[dv,j] + rowsum row via a ones-column appended to V,
  multi^T = heads^T / rowsum, out = multi @ Wo + tokens, LayerNorm rows.

QKV projections run in fp8e4m3 with DoubleRow perf mode (2 K-planes per
matmul): tokens cast to fp8, weights scaled x256 (dodges e4m3 subnormals)
and cast to fp8. The x256 scale cancels: scores pick up 2^16 (folded into
the exp scale) and heads/rowsum both pick up 2^8 (ones column = 256).
Projections are interleaved into the attention stream as hooks so the
Scalar-engine exp stream starts ~10us in instead of after all projections.
Attention math in bf16 matmuls with fp32 PSUM; residual + LN in fp32.
"""

import numpy as np
import ml_dtypes

BF16 = ml_dtypes.bfloat16
FP8 = ml_dtypes.float8_e4m3

B, S, D, H, DK, DV = 4, 2048, 1024, 16, 64, 64
NCORES = 8
NPAIR = 8     # head pairs
NKC = 8       # D // 128 contraction chunks
NIC = 16      # S // 128 i-chunks
JW = 1024     # j columns per core (S/2)
NJCH = 8      # JW // 128
LN_EPS = 1e-5
WSCALE = 256.0  # fp8 weight pre-scale (power of 2)

_CACHE = {}


def _build_nc(apply_affine):
    import concourse.tile as tile
    from concourse import bacc, mybir

    F32 = mybir.dt.float32
    BF = mybir.dt.bfloat16
    F8 = mybir.dt.float8e4
    Exp = mybir.ActivationFunctionType.Exp
    Square = mybir.ActivationFunctionType.Square
    Sqrt = mybir.ActivationFunctionType.Sqrt
    mult = mybir.AluOpType.mult
    add = mybir.AluOpType.add
    DR = mybir.MatmulPerfMode.DoubleRow

    nc = bacc.Bacc(
        "TRN2",
        target_bir_lowering=False,
        debug=False,
        enable_asserts=False,
        num_devices=NCORES,
    )

    # DRAM I/O (per-core views; host prepares layouts)
    tokT_d = nc.dram_tensor("tokT", (128, NKC, S), F8, kind="ExternalInput").ap()
    tokTj_d = nc.dram_tensor("tokTj", (128, NKC, JW), F8, kind="ExternalInput").ap()
    wq_d = nc.dram_tensor("wq", (128, NKC, H * DK), F8, kind="ExternalInput").ap()
    wk_d = nc.dram_tensor("wk", (128, NKC, H * DK), F8, kind="ExternalInput").ap()
    wv_d = nc.dram_tensor("wv", (128, NKC, H * DV), F8, kind="ExternalInput").ap()
    wo_d = nc.dram_tensor("wo", (128, NKC, D), BF, kind="ExternalInput").ap()
    tokres_d = nc.dram_tensor("tokres", (128, NJCH, D), F32, kind="ExternalInput").ap()
    if apply_affine:
        gamma_d = nc.dram_tensor("gamma_bc", (128, D), F32, kind="ExternalInput").ap()
        beta_d = nc.dram_tensor("beta_bc", (128, D), F32, kind="ExternalInput").ap()
    out_d = nc.dram_tensor("out", (128, NJCH, D), F32, kind="ExternalOutput").ap()
    from contextlib import ExitStack

    from concourse.bass import _add_dep_helper

    # Chain all PE matmuls in emission order: stops the scheduler from
    # interleaving row-conflicting matmuls and keeps the stream dense.
    _prev_mm = [None]

    def mm(*args, **kwargs):
        inst = nc.tensor.matmul(*args, **kwargs)
        if _prev_mm[0] is not None:
            _add_dep_helper(inst.ins, _prev_mm[0].ins, sync=False, reason="pe-order")
        _prev_mm[0] = inst
        return inst

    with tile.TileContext(nc) as tc, ExitStack() as stack:
        persist = stack.enter_context(tc.tile_pool(name="persist", bufs=1))
        qT_sb = persist.tile([128, NPAIR, S], BF)          # [pair-dk, pr, i]
        kT_sb = persist.tile([128, NPAIR, JW], BF)         # [pair-dk, pr, j]
        v_sb = persist.tile([128, NIC, H, DV + 1], BF)     # [i%128, ic, h, dv|256s]
        # multi^T stored as one tile per 128-row chunk so out-proj dep tracking
        # stays per-pair (a single big tile serializes on the last DMA write)
        multiT = [
            persist.tile([128, JW], BF, name=f"multiT{kc}") for kc in range(NKC)
        ]
        eps_sb = persist.tile([128, 1], F32)
        ones1_sb = persist.tile([1, DV], F32)
        nc.vector.memset(ones1_sb[:], 1.0)
        if apply_affine:
            gamma_sb = persist.tile([128, D], F32)
            beta_sb = persist.tile([128, D], F32)
            nc.sync.dma_start(gamma_sb[:], gamma_d[:])
            nc.sync.dma_start(beta_sb[:], beta_d[:])
        nc.vector.memset(eps_sb[:], LN_EPS)
        for ic in range(NIC):
            # ones column scaled by WSCALE so rowsum matches the x256 V scale
            nc.vector.memset(v_sb[:, ic, :, DV : DV + 1], WSCALE)

        # pools that outlive pa must be allocated first (LIFO release)
        psS = tc.alloc_tile_pool(name="psS", bufs=2, space="PSUM")
        psAcc = tc.alloc_tile_pool(name="psAcc", bufs=2, space="PSUM")
        pe_pool = stack.enter_context(tc.tile_pool(name="pe", bufs=6))
        pn_pool = stack.enter_context(tc.tile_pool(name="pn", bufs=2))
        pdram = stack.enter_context(tc.tile_pool(name="pdram", bufs=2, space="DRAM"))

        pa = tc.alloc_tile_pool(name="pa", bufs=1)
        wq_sb = pa.tile([128, NKC, H * DK], F8)
        wk_sb = pa.tile([128, NKC, H * DK], F8)
        tokT_sb = pa.tile([128, NKC, S], F8)
        tokTj_sb = pa.tile([128, NKC, JW], F8)
        wv_sb = pa.tile([128, NKC, H * DV], F8)

        # Startup DMA across all three issue paths (SP + ACT HWDGE rings run
        # FIFO independently; gpsimd is SWDGE) so the critical ~3.5MB for the
        # first scores lands in ~1/3 the serial time.
        nc.sync.dma_start(wq_sb[:], wq_d[:])
        for kc in range(NKC):  # tokens i 0..511 first (q chain t0, V chains)
            nc.scalar.dma_start(tokT_sb[:, kc, 0:512], tokT_d[:, kc, 0:512])
        nc.sync.dma_start(wk_sb[:], wk_d[:])
        for kc in range(NKC):
            nc.scalar.dma_start(tokTj_sb[:, kc], tokTj_d[:, kc])
        for kc in range(NKC):
            nc.scalar.dma_start(wv_sb[:, kc], wv_d[:, kc])
        for t in range(1, 4):
            for kc in range(NKC):
                nc.sync.dma_start(
                    tokT_sb[:, kc, t * 512 : (t + 1) * 512],
                    tokT_d[:, kc, t * 512 : (t + 1) * 512],
                )

        def proj_chain(pr, which, t):
            """One 512-wide fp8 DoubleRow projection chain via a borrowed
            scores-pool slot."""
            w_sb, dst, rhs_sb = (
                (wq_sb, qT_sb, tokT_sb) if which == "q" else (wk_sb, kT_sb, tokTj_sb)
            )
            ps = psS.tile([128, 512], F32, tag="sc", name=f"pj{which}{pr}_{t}")
            for kc in range(0, NKC, 2):
                mm(
                    ps[:],
                    w_sb[:, kc : kc + 2, pr * 128 : (pr + 1) * 128],
                    rhs_sb[:, kc : kc + 2, t * 512 : (t + 1) * 512],
                    start=(kc == 0),
                    stop=(kc == NKC - 2),
                    perf_mode=DR,
                )
            nc.vector.tensor_copy(out=dst[:, pr, t * 512 : (t + 1) * 512], in_=ps[:])

        def proj_v(ic):
            """fp8 DoubleRow V projection for one i-chunk via a borrowed
            scores-pool slot."""
            ps = psS.tile([128, 1024], F32, tag="sc", name=f"pjv{ic}")
            for kc in range(0, NKC, 2):
                for nb in range(2):
                    mm(
                        ps[:, nb * 512 : (nb + 1) * 512],
                        tokT_sb[:, kc : kc + 2, ic * 128 : (ic + 1) * 128],
                        wv_sb[:, kc : kc + 2, nb * 512 : (nb + 1) * 512],
                        start=(kc == 0),
                        stop=(kc == NKC - 2),
                        perf_mode=DR,
                    )
            nc.vector.tensor_copy(
                out=v_sb[:, ic, :, 0:DV], in_=ps.rearrange("p (h v) -> p h v", h=H)
            )

        def normalize(pr, acc):
            """multi^T[h] = heads^T / rowsum; runs on DVE/DMA only."""
            for hh in range(2):
                h = 2 * pr + hh
                hraw = pn_pool.tile([DV + 1, JW], F32, tag="hraw", name=f"hraw{h}")
                nc.vector.tensor_copy(out=hraw[:], in_=acc[hh][:])  # frees acc
                rs_dram = pdram.tile([1, JW], F32, tag="rsd", name=f"rsd{h}")
                nc.sync.dma_start(out=rs_dram[:], in_=hraw[DV : DV + 1, :])
                rec_in = pn_pool.tile([DV, JW], F32, tag="rin", name=f"rin{h}")
                nc.gpsimd.dma_start(out=rec_in[:], in_=rs_dram.to_broadcast((DV, JW)))
                nc.vector.reciprocal_approx_fast(out=rec_in[:], in_=rec_in[:])
                if hh == 0:
                    nc.vector.tensor_tensor(
                        multiT[h // 2][0:64, :], hraw[0:DV, :], rec_in[:], mult
                    )
                else:
                    tmp64 = pn_pool.tile([DV, JW], BF, tag="tmp64", name=f"tmp{h}")
                    nc.vector.tensor_tensor(tmp64[:], hraw[0:DV, :], rec_in[:], mult)
                    nc.sync.dma_start(out=multiT[h // 2][64:128, :], in_=tmp64[:])

        def attention(hooks_by_pr, after_pair=None):
            """All pairs, flat: attnV lags one i-chunk behind scores/exp and
            crosses pair boundaries so the exp stream never waits on the PE.
            Hooks and the lag-2 attnV are emitted BEFORE this iteration's
            scores: the scores matmul must wait for an exp to free its PSUM
            slot (ring depth 2), and everything emitted ahead of it in the PE
            chain fills that wait. The attnV lags TWO i-chunks so its eT
            input (the exp output) is long since complete when the PE reaches
            it. hooks_by_pr[pr][ic] is a list of thunks."""
            from collections import deque

            pending = deque()   # (eTs, ic, pr, acc), newest at right
            acc_by_pr = {}

            def do_attnv(peT, pic, ppr, pacc):
                for hh in range(2):
                    for jb in range(2):
                        mm(
                            pacc[hh][:, jb * 512 : (jb + 1) * 512],
                            v_sb[:, pic, 2 * ppr + hh, :],
                            peT[hh][:, jb * 512 : (jb + 1) * 512],
                            start=(pic == 0),
                            stop=(pic == NIC - 1),
                        )

            for pr in range(NPAIR):
                acc_by_pr[pr] = [
                    psAcc.tile([DV + 1, JW], F32, tag="acc", name=f"acc{pr}_{hh}")
                    for hh in range(2)
                ]
                hooks = hooks_by_pr.get(pr, {})
                for ic in range(NIC):
                    for fn in hooks.get(ic, ()):
                        fn()
                    while len(pending) >= 2:
                        pa = pending.popleft()
                        do_attnv(*pa)
                        if pa[1] == NIC - 1:
                            normalize(pa[2], pa[3])
                    ps_s = [
                        psS.tile([128, JW], F32, tag="sc", name=f"ps_s{pr}_{ic}_{hh}")
                        for hh in range(2)
                    ]
                    # scores^T, row-tiled pair (K=64 at partitions 0/64)
                    for hh in range(2):
                        for jb in range(2):
                            mm(
                                ps_s[hh][:, jb * 512 : (jb + 1) * 512],
                                qT_sb[hh * 64 : (hh + 1) * 64, pr, ic * 128 : (ic + 1) * 128],
                                kT_sb[hh * 64 : (hh + 1) * 64, pr, jb * 512 : (jb + 1) * 512],
                                start=True,
                                stop=True,
                            )
                    eTs = []
                    for hh in range(2):
                        eT = pe_pool.tile([128, JW], BF, tag="eT", name=f"eT{pr}_{ic}_{hh}")
                        # x256-scaled q and k: fold 2^-16 into the exp scale
                        nc.scalar.activation(
                            eT[:], ps_s[hh][:], Exp, scale=0.125 / (WSCALE * WSCALE)
                        )
                        eTs.append(eT)
                    pending.append((eTs, ic, pr, acc_by_pr[pr]))
                if after_pair and pr in after_pair:
                    after_pair[pr]()
            while pending:
                pa = pending.popleft()
                do_attnv(*pa)
                if pa[1] == NIC - 1:
                    normalize(pa[2], pa[3])

        # Hook schedule: V projections ride pair 0; each pair p computes its
        # own q chains t1-3 mid-pair and pair p+1's q t0 / k chains late, so
        # every pair's inputs are ready one pair ahead. Pair 7's t1-3 move
        # into pair 6 so tokT dies at pair 6's end (phase C reuses the space).
        hooks = {pr: {} for pr in range(NPAIR)}

        def add_hook(pr, ic, fn):
            hooks[pr].setdefault(ic, []).append(fn)

        for ic in range(2, NIC):
            add_hook(0, ic, lambda ic=ic: proj_v(ic))
        add_hook(0, 1, lambda: proj_v(0))
        add_hook(0, 1, lambda: proj_v(1))
        for pr in range(NPAIR):
            own = pr if pr < 7 else 6
            for t, ic in (
                ((1, 3), (2, 7), (3, 11)) if pr < 7 else ((1, 9), (2, 10), (3, 12))
            ):
                add_hook(own, ic, lambda pr=pr, t=t: proj_chain(pr, "q", t))
            if pr < 7:
                add_hook(pr, 13, lambda pr=pr: proj_chain(pr + 1, "q", 0))
                add_hook(pr, 14, lambda pr=pr: proj_chain(pr + 1, "k", 0))
                add_hook(pr, 15, lambda pr=pr: proj_chain(pr + 1, "k", 1))

        pc_tiles = {}

        def open_phase_c():
            # pa's tensors are dead after pair 6 (pair 7's chains were hoisted
            # into pair 6); reuse the space for phase C inputs so their DMA
            # overlaps pair 7.
            pa.release()
            pc = stack.enter_context(tc.tile_pool(name="pc", bufs=1))
            pc_tiles["wo"] = pc.tile([128, NKC, D], BF, name="wo_sb")
            pc_tiles["tokres"] = pc.tile([128, NJCH, D], F32, name="tokres_sb")
            nc.sync.dma_start(pc_tiles["wo"][:], wo_d[:])
            nc.sync.dma_start(pc_tiles["tokres"][:], tokres_d[:])

        # upfront: just enough projection for pair 0's first scores
        proj_chain(0, "q", 0)
        proj_chain(0, "k", 0)
        proj_chain(0, "k", 1)

        attention(hooks, after_pair={6: open_phase_c})
        wo_sb = pc_tiles["wo"]
        tokres_sb = pc_tiles["tokres"]
        psAcc.release()
        psS.release()
        # ---------------- Phase C: out-proj + residual + LayerNorm ----------------
        with (
            tc.tile_pool(name="pC", bufs=2) as pC,
            tc.tile_pool(name="pStats", bufs=8) as pStats,
            tc.tile_pool(name="psC", bufs=4, space="PSUM") as psC,
        ):
            # Out-proj in two steps per jch: kc 0-6 accumulate early (their
            # multiT chunks are ready pairs before the last normalize), kc 7
            # finishes when multiT[7] lands. Prefilling 4 PSUM groups hides
            # the last normalize's DRAM round-trip behind ~12us of matmuls.
            prefill = {}

            def emit_prefill(jch):
                ps_o = psC.tile([128, D], F32, tag="po", name=f"ps_o{jch}")
                for kc in range(NKC - 1):
                    lhsT = multiT[kc][:, jch * 128 : (jch + 1) * 128]
                    for nb in range(2):
                        mm(
                            ps_o[:, nb * 512 : (nb + 1) * 512],
                            lhsT,
                            wo_sb[:, kc, nb * 512 : (nb + 1) * 512],
                            start=(kc == 0),
                            stop=False,
                        )
                prefill[jch] = ps_o

            for jch in range(4):
                emit_prefill(jch)
            for jch in range(NJCH):
                ps_o = prefill.pop(jch)
                lhsT = multiT[NKC - 1][:, jch * 128 : (jch + 1) * 128]
                for nb in range(2):
                    mm(
                        ps_o[:, nb * 512 : (nb + 1) * 512],
                        lhsT,
                        wo_sb[:, NKC - 1, nb * 512 : (nb + 1) * 512],
                        start=False,
                        stop=True,
                    )
                # x = psum + residual, sum_t = rowsum(x), in one DVE pass
                x_sb = pC.tile([128, D], F32, tag="x", name=f"x{jch}")
                sum_t = pStats.tile([128, 1], F32, tag="sum", name=f"sum{jch}")
                nc.vector.scalar_tensor_tensor(
                    out=x_sb[:],
                    in0=ps_o[:],
                    scalar=0.0,
                    in1=tokres_sb[:, jch, :],
                    op0=add,
                    op1=add,
                    accum_out=sum_t[:],
                )
                negmean = pStats.tile([128, 1], F32, tag="nm", name=f"nm{jch}")
                nc.vector.tensor_scalar_mul(negmean[:], sum_t[:], -1.0 / D)
                # ssq = sum((x-m)^2) on the otherwise-idle Scalar engine:
                # Square(x*1 + negmean) with accum_out
                scrap = pC.tile([128, D], BF, tag="scrap", name=f"scrap{jch}")
                ssq = pStats.tile([128, 1], F32, tag="ssq", name=f"ssq{jch}")
                nc.scalar.activation(
                    scrap[:], x_sb[:], Square, bias=negmean[:], accum_out=ssq[:]
                )
                std_t = pStats.tile([128, 1], F32, tag="std", name=f"std{jch}")
                nc.scalar.activation(std_t[:], ssq[:], Sqrt, bias=eps_sb[:], scale=1.0 / D)
                rstd = pStats.tile([128, 1], F32, tag="rstd", name=f"rstd{jch}")
                nc.vector.reciprocal(rstd[:], std_t[:])
                # (x - m) * rstd == x*rstd + (negmean*rstd), one ACT op
                rstd_nm = pStats.tile([128, 1], F32, tag="rnm", name=f"rnm{jch}")
                nc.vector.tensor_tensor(rstd_nm[:], negmean[:], rstd[:], mult)
                out_sb = pC.tile([128, D], F32, tag="out", name=f"out{jch}")
                nc.scalar.activation(
                    out_sb[:],
                    x_sb[:],
                    mybir.ActivationFunctionType.Identity,
                    bias=rstd_nm[:],
                    scale=rstd[:],
                )
                if apply_affine:
                    nc.gpsimd.tensor_tensor(out_sb[:], out_sb[:], gamma_sb[:], mult)
                    nc.gpsimd.tensor_tensor(out_sb[:], out_sb[:], beta_sb[:], add)
                nc.sync.dma_start(out_d[:, jch], out_sb[:])
                # second prefill wave once the first four STTs are emitted, so
                # the PE chain never parks on a not-yet-freed PSUM group
                if jch == 3:
                    for j2 in range(4, NJCH):
                        emit_prefill(j2)

    nc.compile()
    return nc


def _prep_inputs(tokens, Wq, Wk, Wv, Wo, gamma, beta):
    """Host-side layout prep. Returns per-core input maps."""
    tokens = np.ascontiguousarray(np.asarray(tokens, dtype=np.float32))
    # weights -> [p, kc, n] with row index kc*128+p
    def rows128(a):  # [1024, N] -> [128, 8, N]
        return np.ascontiguousarray(
            a.reshape(NKC, 128, a.shape[-1]).transpose(1, 0, 2)
        )

    wq_all = rows128(
        (np.asarray(Wq).transpose(1, 0, 2).reshape(D, H * DK) * WSCALE).astype(FP8)
    )
    wk_all = rows128(
        (np.asarray(Wk).transpose(1, 0, 2).reshape(D, H * DK) * WSCALE).astype(FP8)
    )
    wv_all = rows128(
        (np.asarray(Wv).transpose(1, 0, 2).reshape(D, H * DV) * WSCALE).astype(FP8)
    )
    wo_all = rows128(np.asarray(Wo).astype(BF16))
    gamma_bc = np.ascontiguousarray(
        np.broadcast_to(np.asarray(gamma, np.float32), (128, D))
    )
    beta_bc = np.ascontiguousarray(
        np.broadcast_to(np.asarray(beta, np.float32), (128, D))
    )

    tokT_by_b = []
    for b in range(B):
        tokT_by_b.append(rows128(tokens[b].T.astype(FP8)))  # [128, 8, 2048]

    in_maps = []
    for c in range(NCORES):
        b, jc = c // 2, c % 2
        tokT = tokT_by_b[b]
        tokTj = np.ascontiguousarray(tokT[:, :, jc * JW : (jc + 1) * JW])
        tokres = np.ascontiguousarray(
            tokens[b, jc * JW : (jc + 1) * JW]
            .reshape(NJCH, 128, D)
            .transpose(1, 0, 2)
        )
        in_maps.append(
            {
                "tokT": tokT,
                "tokTj": tokTj,
                "wq": wq_all,
                "wk": wk_all,
                "wv": wv_all,
                "wo": wo_all,
                "tokres": tokres,
                "gamma_bc": gamma_bc,
                "beta_bc": beta_bc,
            }
        )
    return in_maps


def run(inputs, trace=False, tmpdir=None):
    """Run on hardware; returns (output, BassKernelResults)."""
    from concourse.bass_utils import run_bass_kernel_spmd

    apply_affine = not (
        np.all(np.asarray(inputs["gamma"]) == 1.0)
        and np.all(np.asarray(inputs["beta"]) == 0.0)
    )
    key = ("nc", apply_affine)
    if key not in _CACHE:
        _CACHE[key] = _build_nc(apply_affine)
    nc = _CACHE[key]
    in_maps = _prep_inputs(**inputs)
    res = run_bass_kernel_spmd(
        nc, in_maps, core_ids=list(range(NCORES)), trace=trace, tmpdir=tmpdir
    )
    out = np.empty((B, S, D), np.float32)
    for c in range(NCORES):
        b, jc = c // 2, c % 2
        o = res.results[c]["out"]  # [128, 8, 1024]
        out[b, jc * JW : (jc + 1) * JW] = (
            o.transpose(1, 0, 2).reshape(JW, D)
        )
    return out, res


def kernel(tokens, Wq, Wk, Wv, Wo, gamma, beta):
    out, _ = run(
        dict(tokens=tokens, Wq=Wq, Wk=Wk, Wv=Wv, Wo=Wo, gamma=gamma, beta=beta)
    )
    return out


# revision 24
# speedup vs baseline: 1.3264x; 1.0528x over previous
"""TRN2 Bass kernel for fused MHA (softmax-over-query quirk) + out-proj + residual + LayerNorm.

Problem shapes (hardcoded): tokens [4,2048,1024], Wq/Wk [16,1024,64], Wv [16,1024,64],
Wo [1024,1024], gamma/beta [1024]. Output [4,2048,1024] fp32.

Sharding: 8 cores, core c owns (batch b=c//2, S-half jc=c%2) of the OUTPUT rows.
No collectives. Each core computes, for its batch b:
  qT[dk,i] (full S), kT[dk,j] (its half), V[i,dv] (full S) in bf16,
  scores^T[i,j] = q_i.k_j (PSUM fp32), e = exp(scores/8) (bf16),
  heads^T[dv,j] + rowsum row via a ones-column appended to V,
  multi^T = heads^T / rowsum, out = multi @ Wo + tokens, LayerNorm rows.

QKV projections run in fp8e4m3 with DoubleRow perf mode (2 K-planes per
matmul): tokens cast to fp8, weights scaled x256 (dodges e4m3 subnormals)
and cast to fp8. The x256 scale cancels: scores pick up 2^16 (folded into
the exp scale) and heads/rowsum both pick up 2^8 (ones column = 256).
Projections are interleaved into the attention stream as hooks so the
Scalar-engine exp stream starts ~10us in instead of after all projections.
Attention math in bf16 matmuls with fp32 PSUM; residual + LN in fp32.
"""

import numpy as np
import ml_dtypes

BF16 = ml_dtypes.bfloat16
FP8 = ml_dtypes.float8_e4m3

B, S, D, H, DK, DV = 4, 2048, 1024, 16, 64, 64
NCORES = 8
NPAIR = 8     # head pairs
NKC = 8       # D // 128 contraction chunks
NIC = 16      # S // 128 i-chunks
JW = 1024     # j columns per core (S/2)
NJCH = 8      # JW // 128
LN_EPS = 1e-5
WSCALE = 256.0  # fp8 weight pre-scale (power of 2)

_CACHE = {}


def _build_nc(apply_affine):
    import concourse.tile as tile
    from concourse import bacc, mybir

    F32 = mybir.dt.float32
    BF = mybir.dt.bfloat16
    F8 = mybir.dt.float8e4
    Exp = mybir.ActivationFunctionType.Exp
    Square = mybir.ActivationFunctionType.Square
    Sqrt = mybir.ActivationFunctionType.Sqrt
    mult = mybir.AluOpType.mult
    add = mybir.AluOpType.add
    DR = mybir.MatmulPerfMode.DoubleRow

    nc = bacc.Bacc(
        "TRN2",
        target_bir_lowering=False,
        debug=False,
        enable_asserts=False,
        num_devices=NCORES,
    )

    # DRAM I/O (per-core views; host prepares layouts)
    tokT_d = nc.dram_tensor("tokT", (128, NKC, S), F8, kind="ExternalInput").ap()
    tokTj_d = nc.dram_tensor("tokTj", (128, NKC, JW), F8, kind="ExternalInput").ap()
    wq_d = nc.dram_tensor("wq", (128, NKC, H * DK), F8, kind="ExternalInput").ap()
    wk_d = nc.dram_tensor("wk", (128, NKC, H * DK), F8, kind="ExternalInput").ap()
    wv_d = nc.dram_tensor("wv", (128, NKC, H * DV), F8, kind="ExternalInput").ap()
    wo_d = nc.dram_tensor("wo", (128, NKC, D), BF, kind="ExternalInput").ap()
    tokres_d = nc.dram_tensor("tokres", (128, NJCH, D), F32, kind="ExternalInput").ap()
    if apply_affine:
        gamma_d = nc.dram_tensor("gamma_bc", (128, D), F32, kind="ExternalInput").ap()
        beta_d = nc.dram_tensor("beta_bc", (128, D), F32, kind="ExternalInput").ap()
    out_d = nc.dram_tensor("out", (128, NJCH, D), F32, kind="ExternalOutput").ap()
    from contextlib import ExitStack

    from concourse.bass import _add_dep_helper

    # Chain all PE matmuls in emission order: stops the scheduler from
    # interleaving row-conflicting matmuls and keeps the stream dense.
    _prev_mm = [None]

    def mm(*args, **kwargs):
        inst = nc.tensor.matmul(*args, **kwargs)
        if _prev_mm[0] is not None:
            _add_dep_helper(inst.ins, _prev_mm[0].ins, sync=False, reason="pe-order")
        _prev_mm[0] = inst
        return inst

    with tile.TileContext(nc) as tc, ExitStack() as stack:
        persist = stack.enter_context(tc.tile_pool(name="persist", bufs=1))
        qT_sb = persist.tile([128, NPAIR, S], BF)          # [pair-dk, pr, i]
        kT_sb = persist.tile([128, NPAIR, JW], BF)         # [pair-dk, pr, j]
        v_sb = persist.tile([128, NIC, H, DV + 1], BF)     # [i%128, ic, h, dv|256s]
        # multi^T stored as one tile per 128-row chunk so out-proj dep tracking
        # stays per-pair (a single big tile serializes on the last DMA write)
        multiT = [
            persist.tile([128, JW], BF, name=f"multiT{kc}") for kc in range(NKC)
        ]
        eps_sb = persist.tile([128, 1], F32)
        ones1_sb = persist.tile([1, DV], F32)
        nc.vector.memset(ones1_sb[:], 1.0)
        if apply_affine:
            gamma_sb = persist.tile([128, D], F32)
            beta_sb = persist.tile([128, D], F32)
            nc.sync.dma_start(gamma_sb[:], gamma_d[:])
            nc.sync.dma_start(beta_sb[:], beta_d[:])
        nc.vector.memset(eps_sb[:], LN_EPS)
        for ic in range(NIC):
            # ones column scaled by WSCALE so rowsum matches the x256 V scale
            nc.vector.memset(v_sb[:, ic, :, DV : DV + 1], WSCALE)

        # pools that outlive pa must be allocated first (LIFO release)
        # psS: four 1-bank [128,512] slots. Depth 4 (vs 2x [128,1024]) means a
        # scores matmul waits on an exp from ~2 i-chunks back, so projection
        # hooks borrowing a slot no longer stall the exp stream.
        psS = tc.alloc_tile_pool(name="psS", bufs=4, space="PSUM")
        psAcc = tc.alloc_tile_pool(name="psAcc", bufs=2, space="PSUM")
        pe_pool = stack.enter_context(tc.tile_pool(name="pe", bufs=12))
        pn_pool = stack.enter_context(tc.tile_pool(name="pn", bufs=2))
        pdram = stack.enter_context(tc.tile_pool(name="pdram", bufs=2, space="DRAM"))

        pa = tc.alloc_tile_pool(name="pa", bufs=1)
        wq_sb = pa.tile([128, NKC, H * DK], F8)
        wk_sb = pa.tile([128, NKC, H * DK], F8)
        tokT_sb = pa.tile([128, NKC, S], F8)
        tokTj_sb = pa.tile([128, NKC, JW], F8)
        wv_sb = pa.tile([128, NKC, H * DV], F8)

        # Startup DMA over both HWDGE rings (SP + ACT, independent FIFOs).
        # Critical path to the first exp is only the pair-0 weight slices +
        # tokens-i0 + tokTj; everything else queues behind.
        nc.sync.dma_start(wq_sb[:, :, 0:128], wq_d[:, :, 0:128])
        nc.sync.dma_start(wk_sb[:, :, 0:128], wk_d[:, :, 0:128])
        for kc in range(NKC):  # tokens i 0..511 first (q chain t0, V chains)
            nc.scalar.dma_start(tokT_sb[:, kc, 0:512], tokT_d[:, kc, 0:512])
        for kc in range(NKC):
            nc.scalar.dma_start(tokTj_sb[:, kc], tokTj_d[:, kc])
        for t in range(1, 4):
            for kc in range(NKC):
                nc.sync.dma_start(
                    tokT_sb[:, kc, t * 512 : (t + 1) * 512],
                    tokT_d[:, kc, t * 512 : (t + 1) * 512],
                )
        for kc in range(NKC):
            nc.scalar.dma_start(wv_sb[:, kc], wv_d[:, kc])
        nc.sync.dma_start(wq_sb[:, :, 128:], wq_d[:, :, 128:])
        nc.sync.dma_start(wk_sb[:, :, 128:], wk_d[:, :, 128:])

        def proj_chain(pr, which, t):
            """One 512-wide fp8 DoubleRow projection chain via a borrowed
            scores-pool slot."""
            w_sb, dst, rhs_sb = (
                (wq_sb, qT_sb, tokT_sb) if which == "q" else (wk_sb, kT_sb, tokTj_sb)
            )
            ps = psS.tile([128, 512], F32, tag="sc", name=f"pj{which}{pr}_{t}")
            for kc in range(0, NKC, 2):
                mm(
                    ps[:],
                    w_sb[:, kc : kc + 2, pr * 128 : (pr + 1) * 128],
                    rhs_sb[:, kc : kc + 2, t * 512 : (t + 1) * 512],
                    start=(kc == 0),
                    stop=(kc == NKC - 2),
                    perf_mode=DR,
                )
            nc.vector.tensor_copy(out=dst[:, pr, t * 512 : (t + 1) * 512], in_=ps[:])

        def proj_v(ic):
            """fp8 DoubleRow V projection for one i-chunk via two borrowed
            scores-pool slots (one per 8-head half)."""
            for nb in range(2):
                ps = psS.tile([128, 512], F32, tag="sc", name=f"pjv{ic}_{nb}")
                for kc in range(0, NKC, 2):
                    mm(
                        ps[:],
                        tokT_sb[:, kc : kc + 2, ic * 128 : (ic + 1) * 128],
                        wv_sb[:, kc : kc + 2, nb * 512 : (nb + 1) * 512],
                        start=(kc == 0),
                        stop=(kc == NKC - 2),
                        perf_mode=DR,
                    )
                nc.vector.tensor_copy(
                    out=v_sb[:, ic, nb * 8 : (nb + 1) * 8, 0:DV],
                    in_=ps.rearrange("p (h v) -> p h v", h=8),
                )

        def normalize(pr, acc):
            """multi^T[h] = heads^T / rowsum; runs on DVE/DMA only."""
            for hh in range(2):
                h = 2 * pr + hh
                hraw = pn_pool.tile([DV + 1, JW], F32, tag="hraw", name=f"hraw{h}")
                nc.vector.tensor_copy(out=hraw[:], in_=acc[hh][:])  # frees acc
                rs_dram = pdram.tile([1, JW], F32, tag="rsd", name=f"rsd{h}")
                nc.sync.dma_start(out=rs_dram[:], in_=hraw[DV : DV + 1, :])
                rec_in = pn_pool.tile([DV, JW], F32, tag="rin", name=f"rin{h}")
                nc.gpsimd.dma_start(out=rec_in[:], in_=rs_dram.to_broadcast((DV, JW)))
                nc.vector.reciprocal_approx_fast(out=rec_in[:], in_=rec_in[:])
                if hh == 0:
                    nc.vector.tensor_tensor(
                        multiT[h // 2][0:64, :], hraw[0:DV, :], rec_in[:], mult
                    )
                else:
                    tmp64 = pn_pool.tile([DV, JW], BF, tag="tmp64", name=f"tmp{h}")
                    nc.vector.tensor_tensor(tmp64[:], hraw[0:DV, :], rec_in[:], mult)
                    nc.sync.dma_start(out=multiT[h // 2][64:128, :], in_=tmp64[:])

        def attention(hooks_by_pr, after_pair=None):
            """All pairs, flat: attnV lags one i-chunk behind scores/exp and
            crosses pair boundaries so the exp stream never waits on the PE.
            Hooks and the lag-2 attnV are emitted BEFORE this iteration's
            scores: the scores matmul must wait for an exp to free its PSUM
            slot (ring depth 2), and everything emitted ahead of it in the PE
            chain fills that wait. The attnV lags TWO i-chunks so its eT
            input (the exp output) is long since complete when the PE reaches
            it. hooks_by_pr[pr][ic] is a list of thunks."""
            from collections import deque

            pending = deque()   # (eTs, ic, pr, acc), newest at right
            acc_by_pr = {}

            def do_attnv(peT, pic, ppr, pacc):
                for hh in range(2):
                    for jb in range(2):
                        mm(
                            pacc[hh][:, jb * 512 : (jb + 1) * 512],
                            v_sb[:, pic, 2 * ppr + hh, :],
                            peT[2 * hh + jb][:],
                            start=(pic == 0),
                            stop=(pic == NIC - 1),
                        )

            for pr in range(NPAIR):
                acc_by_pr[pr] = [
                    psAcc.tile([DV + 1, JW], F32, tag="acc", name=f"acc{pr}_{hh}")
                    for hh in range(2)
                ]
                hooks = hooks_by_pr.get(pr, {})
                for ic in range(NIC):
                    for fn in hooks.get(ic, ()):
                        fn()
                    while len(pending) >= 2:
                        pa = pending.popleft()
                        do_attnv(*pa)
                        if pa[1] == NIC - 1:
                            normalize(pa[2], pa[3])
                    # scores^T, row-tiled pair (K=64 at partitions 0/64); one
                    # 1-bank PSUM slot and one exp per (hh, jb) quarter
                    eTs = []
                    for hh in range(2):
                        for jb in range(2):
                            ps_s = psS.tile(
                                [128, 512], F32, tag="sc",
                                name=f"ps_s{pr}_{ic}_{hh}{jb}",
                            )
                            mm(
                                ps_s[:],
                                qT_sb[hh * 64 : (hh + 1) * 64, pr, ic * 128 : (ic + 1) * 128],
                                kT_sb[hh * 64 : (hh + 1) * 64, pr, jb * 512 : (jb + 1) * 512],
                                start=True,
                                stop=True,
                            )
                            eT = pe_pool.tile(
                                [128, 512], BF, tag="eT", name=f"eT{pr}_{ic}_{hh}{jb}"
                            )
                            # x256-scaled q and k: fold 2^-16 into the exp scale
                            nc.scalar.activation(
                                eT[:], ps_s[:], Exp, scale=0.125 / (WSCALE * WSCALE)
                            )
                            eTs.append(eT)
                    pending.append((eTs, ic, pr, acc_by_pr[pr]))
                if after_pair and pr in after_pair:
                    after_pair[pr]()
            while pending:
                pa = pending.popleft()
                do_attnv(*pa)
                if pa[1] == NIC - 1:
                    normalize(pa[2], pa[3])

        # Hook schedule: V projections ride pair 0; each pair p computes its
        # own q chains t1-3 mid-pair and pair p+1's q t0 / k chains late, so
        # every pair's inputs are ready one pair ahead. Pair 7's t1-3 move
        # into pair 6 so tokT dies at pair 6's end (phase C reuses the space).
        hooks = {pr: {} for pr in range(NPAIR)}

        def add_hook(pr, ic, fn):
            hooks[pr].setdefault(ic, []).append(fn)

        for ic in range(2, NIC):
            add_hook(0, ic, lambda ic=ic: proj_v(ic))
        add_hook(0, 1, lambda: proj_v(0))
        add_hook(0, 1, lambda: proj_v(1))
        for pr in range(NPAIR):
            own = pr if pr < 7 else 6
            for t, ic in (
                ((1, 3), (2, 7), (3, 11)) if pr < 7 else ((1, 9), (2, 10), (3, 12))
            ):
                add_hook(own, ic, lambda pr=pr, t=t: proj_chain(pr, "q", t))
            if pr < 7:
                add_hook(pr, 13, lambda pr=pr: proj_chain(pr + 1, "q", 0))
                add_hook(pr, 14, lambda pr=pr: proj_chain(pr + 1, "k", 0))
                add_hook(pr, 15, lambda pr=pr: proj_chain(pr + 1, "k", 1))

        pc_tiles = {}

        def open_phase_c():
            # pa's tensors are dead after pair 6 (pair 7's chains were hoisted
            # into pair 6); reuse the space for phase C inputs so their DMA
            # overlaps pair 7.
            pa.release()
            pc = stack.enter_context(tc.tile_pool(name="pc", bufs=1))
            pc_tiles["wo"] = pc.tile([128, NKC, D], BF, name="wo_sb")
            pc_tiles["tokres"] = pc.tile([128, NJCH, D], F32, name="tokres_sb")
            nc.sync.dma_start(pc_tiles["wo"][:], wo_d[:])
            nc.sync.dma_start(pc_tiles["tokres"][:], tokres_d[:])

        # upfront: just enough projection for pair 0's first scores
        proj_chain(0, "q", 0)
        proj_chain(0, "k", 0)
        proj_chain(0, "k", 1)

        attention(hooks, after_pair={6: open_phase_c})
        wo_sb = pc_tiles["wo"]
        tokres_sb = pc_tiles["tokres"]
        psAcc.release()
        psS.release()
        # ---------------- Phase C: out-proj + residual + LayerNorm ----------------
        with (
            tc.tile_pool(name="pC", bufs=2) as pC,
            tc.tile_pool(name="pStats", bufs=8) as pStats,
            tc.tile_pool(name="psC", bufs=4, space="PSUM") as psC,
        ):
            # Out-proj in two steps per jch: kc 0-6 accumulate early (their
            # multiT chunks are ready pairs before the last normalize), kc 7
            # finishes when multiT[7] lands. Prefilling 4 PSUM groups hides
            # the last normalize's DRAM round-trip behind ~12us of matmuls.
            prefill = {}

            def emit_prefill(jch):
                ps_o = psC.tile([128, D], F32, tag="po", name=f"ps_o{jch}")
                for kc in range(NKC - 1):
                    lhsT = multiT[kc][:, jch * 128 : (jch + 1) * 128]
                    for nb in range(2):
                        mm(
                            ps_o[:, nb * 512 : (nb + 1) * 512],
                            lhsT,
                            wo_sb[:, kc, nb * 512 : (nb + 1) * 512],
                            start=(kc == 0),
                            stop=False,
                        )
                prefill[jch] = ps_o

            for jch in range(4):
                emit_prefill(jch)
            for jch in range(NJCH):
                ps_o = prefill.pop(jch)
                lhsT = multiT[NKC - 1][:, jch * 128 : (jch + 1) * 128]
                for nb in range(2):
                    mm(
                        ps_o[:, nb * 512 : (nb + 1) * 512],
                        lhsT,
                        wo_sb[:, NKC - 1, nb * 512 : (nb + 1) * 512],
                        start=False,
                        stop=True,
                    )
                # x = psum + residual, sum_t = rowsum(x), in one DVE pass
                x_sb = pC.tile([128, D], F32, tag="x", name=f"x{jch}")
                sum_t = pStats.tile([128, 1], F32, tag="sum", name=f"sum{jch}")
                nc.vector.scalar_tensor_tensor(
                    out=x_sb[:],
                    in0=ps_o[:],
                    scalar=0.0,
                    in1=tokres_sb[:, jch, :],
                    op0=add,
                    op1=add,
                    accum_out=sum_t[:],
                )
                negmean = pStats.tile([128, 1], F32, tag="nm", name=f"nm{jch}")
                nc.vector.tensor_scalar_mul(negmean[:], sum_t[:], -1.0 / D)
                # ssq = sum((x-m)^2) on the otherwise-idle Scalar engine:
                # Square(x*1 + negmean) with accum_out
                scrap = pC.tile([128, D], BF, tag="scrap", name=f"scrap{jch}")
                ssq = pStats.tile([128, 1], F32, tag="ssq", name=f"ssq{jch}")
                nc.scalar.activation(
                    scrap[:], x_sb[:], Square, bias=negmean[:], accum_out=ssq[:]
                )
                std_t = pStats.tile([128, 1], F32, tag="std", name=f"std{jch}")
                nc.scalar.activation(std_t[:], ssq[:], Sqrt, bias=eps_sb[:], scale=1.0 / D)
                rstd = pStats.tile([128, 1], F32, tag="rstd", name=f"rstd{jch}")
                nc.vector.reciprocal(rstd[:], std_t[:])
                # (x - m) * rstd == x*rstd + (negmean*rstd), one ACT op
                rstd_nm = pStats.tile([128, 1], F32, tag="rnm", name=f"rnm{jch}")
                nc.vector.tensor_tensor(rstd_nm[:], negmean[:], rstd[:], mult)
                out_sb = pC.tile([128, D], F32, tag="out", name=f"out{jch}")
                nc.scalar.activation(
                    out_sb[:],
                    x_sb[:],
                    mybir.ActivationFunctionType.Identity,
                    bias=rstd_nm[:],
                    scale=rstd[:],
                )
                if apply_affine:
                    nc.gpsimd.tensor_tensor(out_sb[:], out_sb[:], gamma_sb[:], mult)
                    nc.gpsimd.tensor_tensor(out_sb[:], out_sb[:], beta_sb[:], add)
                nc.sync.dma_start(out_d[:, jch], out_sb[:])
                # second prefill wave once the first four STTs are emitted, so
                # the PE chain never parks on a not-yet-freed PSUM group
                if jch == 3:
                    for j2 in range(4, NJCH):
                        emit_prefill(j2)

    nc.compile()
    return nc


def _prep_inputs(tokens, Wq, Wk, Wv, Wo, gamma, beta):
    """Host-side layout prep. Returns per-core input maps."""
    tokens = np.ascontiguousarray(np.asarray(tokens, dtype=np.float32))
    # weights -> [p, kc, n] with row index kc*128+p
    def rows128(a):  # [1024, N] -> [128, 8, N]
        return np.ascontiguousarray(
            a.reshape(NKC, 128, a.shape[-1]).transpose(1, 0, 2)
        )

    wq_all = rows128(
        (np.asarray(Wq).transpose(1, 0, 2).reshape(D, H * DK) * WSCALE).astype(FP8)
    )
    wk_all = rows128(
        (np.asarray(Wk).transpose(1, 0, 2).reshape(D, H * DK) * WSCALE).astype(FP8)
    )
    wv_all = rows128(
        (np.asarray(Wv).transpose(1, 0, 2).reshape(D, H * DV) * WSCALE).astype(FP8)
    )
    wo_all = rows128(np.asarray(Wo).astype(BF16))
    gamma_bc = np.ascontiguousarray(
        np.broadcast_to(np.asarray(gamma, np.float32), (128, D))
    )
    beta_bc = np.ascontiguousarray(
        np.broadcast_to(np.asarray(beta, np.float32), (128, D))
    )

    tokT_by_b = []
    for b in range(B):
        tokT_by_b.append(rows128(tokens[b].T.astype(FP8)))  # [128, 8, 2048]

    in_maps = []
    for c in range(NCORES):
        b, jc = c // 2, c % 2
        tokT = tokT_by_b[b]
        tokTj = np.ascontiguousarray(tokT[:, :, jc * JW : (jc + 1) * JW])
        tokres = np.ascontiguousarray(
            tokens[b, jc * JW : (jc + 1) * JW]
            .reshape(NJCH, 128, D)
            .transpose(1, 0, 2)
        )
        in_maps.append(
            {
                "tokT": tokT,
                "tokTj": tokTj,
                "wq": wq_all,
                "wk": wk_all,
                "wv": wv_all,
                "wo": wo_all,
                "tokres": tokres,
                "gamma_bc": gamma_bc,
                "beta_bc": beta_bc,
            }
        )
    return in_maps


def run(inputs, trace=False, tmpdir=None):
    """Run on hardware; returns (output, BassKernelResults)."""
    from concourse.bass_utils import run_bass_kernel_spmd

    apply_affine = not (
        np.all(np.asarray(inputs["gamma"]) == 1.0)
        and np.all(np.asarray(inputs["beta"]) == 0.0)
    )
    key = ("nc", apply_affine)
    if key not in _CACHE:
        _CACHE[key] = _build_nc(apply_affine)
    nc = _CACHE[key]
    in_maps = _prep_inputs(**inputs)
    res = run_bass_kernel_spmd(
        nc, in_maps, core_ids=list(range(NCORES)), trace=trace, tmpdir=tmpdir
    )
    out = np.empty((B, S, D), np.float32)
    for c in range(NCORES):
        b, jc = c // 2, c % 2
        o = res.results[c]["out"]  # [128, 8, 1024]
        out[b, jc * JW : (jc + 1) * JW] = (
            o.transpose(1, 0, 2).reshape(JW, D)
        )
    return out, res


def kernel(tokens, Wq, Wk, Wv, Wo, gamma, beta):
    out, _ = run(
        dict(tokens=tokens, Wq=Wq, Wk=Wk, Wv=Wv, Wo=Wo, gamma=gamma, beta=beta)
    )
    return out


# revision 29
# speedup vs baseline: 1.3658x; 1.0296x over previous
"""TRN2 Bass kernel for fused MHA (softmax-over-query quirk) + out-proj + residual + LayerNorm.

Problem shapes (hardcoded): tokens [4,2048,1024], Wq/Wk [16,1024,64], Wv [16,1024,64],
Wo [1024,1024], gamma/beta [1024]. Output [4,2048,1024] fp32.

Sharding: 8 cores, core c owns (batch b=c//2, S-half jc=c%2) of the OUTPUT rows.
No collectives. Each core computes, for its batch b:
  qT[dk,i] (full S), kT[dk,j] (its half), V[i,dv] (full S) in bf16,
  scores^T[i,j] = q_i.k_j (PSUM fp32), e = exp(scores/8) (bf16),
  heads^T[dv,j] + rowsum row via a ones-column appended to V,
  multi^T = heads^T / rowsum, out = multi @ Wo + tokens, LayerNorm rows.

QKV projections run in fp8e4m3 with DoubleRow perf mode (2 K-planes per
matmul): tokens cast to fp8, weights scaled x256 (dodges e4m3 subnormals)
and cast to fp8. The x256 scale cancels: scores pick up 2^16 (folded into
the exp scale) and heads/rowsum both pick up 2^8 (ones column = 256).
Projections are interleaved into the attention stream as hooks so the
Scalar-engine exp stream starts ~10us in instead of after all projections.
Attention math in bf16 matmuls with fp32 PSUM; residual + LN in fp32.
"""

import numpy as np
import ml_dtypes

BF16 = ml_dtypes.bfloat16
FP8 = ml_dtypes.float8_e4m3

B, S, D, H, DK, DV = 4, 2048, 1024, 16, 64, 64
NCORES = 8
NPAIR = 8     # head pairs
NKC = 8       # D // 128 contraction chunks
NIC = 16      # S // 128 i-chunks
JW = 1024     # j columns per core (S/2)
NJCH = 8      # JW // 128
LN_EPS = 1e-5
WSCALE = 256.0  # fp8 weight pre-scale (power of 2)

_CACHE = {}


def _build_nc(apply_affine):
    import concourse.tile as tile
    from concourse import bacc, mybir

    F32 = mybir.dt.float32
    BF = mybir.dt.bfloat16
    F8 = mybir.dt.float8e4
    Exp = mybir.ActivationFunctionType.Exp
    Square = mybir.ActivationFunctionType.Square
    Sqrt = mybir.ActivationFunctionType.Sqrt
    mult = mybir.AluOpType.mult
    add = mybir.AluOpType.add
    DR = mybir.MatmulPerfMode.DoubleRow

    nc = bacc.Bacc(
        "TRN2",
        target_bir_lowering=False,
        debug=False,
        enable_asserts=False,
        num_devices=NCORES,
    )

    # DRAM I/O (per-core views; host prepares layouts)
    tokT_d = nc.dram_tensor("tokT", (128, NKC, S), F8, kind="ExternalInput").ap()
    tokTj_d = nc.dram_tensor("tokTj", (128, NKC, JW), F8, kind="ExternalInput").ap()
    wq_d = nc.dram_tensor("wq", (128, NKC, H * DK), F8, kind="ExternalInput").ap()
    wk_d = nc.dram_tensor("wk", (128, NKC, H * DK), F8, kind="ExternalInput").ap()
    wv_d = nc.dram_tensor("wv", (128, NKC, H * DV), F8, kind="ExternalInput").ap()
    wo_d = nc.dram_tensor("wo", (128, NKC, D), F8, kind="ExternalInput").ap()
    tokres_d = nc.dram_tensor("tokres", (128, NJCH, D), F32, kind="ExternalInput").ap()
    if apply_affine:
        gamma_d = nc.dram_tensor("gamma_bc", (128, D), F32, kind="ExternalInput").ap()
        beta_d = nc.dram_tensor("beta_bc", (128, D), F32, kind="ExternalInput").ap()
    out_d = nc.dram_tensor("out", (128, NJCH, D), F32, kind="ExternalOutput").ap()
    from contextlib import ExitStack

    from concourse.bass import _add_dep_helper

    # Chain all PE matmuls in emission order: stops the scheduler from
    # interleaving row-conflicting matmuls and keeps the stream dense.
    _prev_mm = [None]

    def mm(*args, **kwargs):
        inst = nc.tensor.matmul(*args, **kwargs)
        if _prev_mm[0] is not None:
            _add_dep_helper(inst.ins, _prev_mm[0].ins, sync=False, reason="pe-order")
        _prev_mm[0] = inst
        return inst

    with tile.TileContext(nc) as tc, ExitStack() as stack:
        persist = stack.enter_context(tc.tile_pool(name="persist", bufs=1))
        qT_sb = persist.tile([128, NPAIR, S], BF)          # [pair-dk, pr, i]
        kT_sb = persist.tile([128, NPAIR, JW], BF)         # [pair-dk, pr, j]
        v_sb = persist.tile([128, NIC, H, DV + 1], BF)     # [i%128, ic, h, dv|256s]
        # multi^T in fp8, one tile per KC-PAIR ([128, 2, JW], middle dim = the
        # DoubleRow K-plane) so the out-proj runs fp8 DoubleRow; per-pair-ish
        # tiles keep out-proj dep tracking from serializing on the last write
        multiT = [
            persist.tile([128, 2, JW], F8, name=f"multiT{i}") for i in range(NKC // 2)
        ]
        eps_sb = persist.tile([128, 1], F32)
        ones1_sb = persist.tile([1, DV], F32)
        nc.vector.memset(ones1_sb[:], 1.0)
        if apply_affine:
            gamma_sb = persist.tile([128, D], F32)
            beta_sb = persist.tile([128, D], F32)
            nc.sync.dma_start(gamma_sb[:], gamma_d[:])
            nc.sync.dma_start(beta_sb[:], beta_d[:])
        nc.vector.memset(eps_sb[:], LN_EPS)
        for ic in range(NIC):
            # ones column scaled by WSCALE so rowsum matches the x256 V scale
            nc.vector.memset(v_sb[:, ic, :, DV : DV + 1], WSCALE)

        # pools that outlive pa must be allocated first (LIFO release)
        # psS: four 1-bank [128,512] slots. Depth 4 (vs 2x [128,1024]) means a
        # scores matmul waits on an exp from ~2 i-chunks back, so projection
        # hooks borrowing a slot no longer stall the exp stream.
        psS = tc.alloc_tile_pool(name="psS", bufs=4, space="PSUM")
        psAcc = tc.alloc_tile_pool(name="psAcc", bufs=2, space="PSUM")
        pe_pool = stack.enter_context(tc.tile_pool(name="pe", bufs=12))
        pn_pool = stack.enter_context(tc.tile_pool(name="pn", bufs=2))
        pdram = stack.enter_context(tc.tile_pool(name="pdram", bufs=2, space="DRAM"))

        pa = tc.alloc_tile_pool(name="pa", bufs=1)
        wq_sb = pa.tile([128, NKC, H * DK], F8)
        wk_sb = pa.tile([128, NKC, H * DK], F8)
        tokT_sb = pa.tile([128, NKC, S], F8)
        tokTj_sb = pa.tile([128, NKC, JW], F8)
        wv_sb = pa.tile([128, NKC, H * DV], F8)

        # Startup DMA over both HWDGE rings (SP + ACT, independent FIFOs).
        # Critical path to the first exp is only the pair-0 weight slices +
        # tokens-i0 + tokTj; everything else queues behind.
        nc.sync.dma_start(wq_sb[:, :, 0:128], wq_d[:, :, 0:128])
        nc.sync.dma_start(wk_sb[:, :, 0:128], wk_d[:, :, 0:128])
        # tokens i 0..511 first (q chain t0, V chains), as single 3D transfers
        nc.scalar.dma_start(tokT_sb[:, :, 0:512], tokT_d[:, :, 0:512])
        nc.scalar.dma_start(tokTj_sb[:], tokTj_d[:])
        nc.sync.dma_start(tokT_sb[:, :, 512:1024], tokT_d[:, :, 512:1024])
        nc.scalar.dma_start(wv_sb[:], wv_d[:])
        nc.sync.dma_start(tokT_sb[:, :, 1024:2048], tokT_d[:, :, 1024:2048])
        nc.sync.dma_start(wq_sb[:, :, 128:], wq_d[:, :, 128:])
        nc.sync.dma_start(wk_sb[:, :, 128:], wk_d[:, :, 128:])

        def proj_chain(pr, which, t):
            """One 512-wide fp8 DoubleRow projection chain via a borrowed
            scores-pool slot."""
            w_sb, dst, rhs_sb = (
                (wq_sb, qT_sb, tokT_sb) if which == "q" else (wk_sb, kT_sb, tokTj_sb)
            )
            ps = psS.tile([128, 512], F32, tag="sc", name=f"pj{which}{pr}_{t}")
            for kc in range(0, NKC, 2):
                mm(
                    ps[:],
                    w_sb[:, kc : kc + 2, pr * 128 : (pr + 1) * 128],
                    rhs_sb[:, kc : kc + 2, t * 512 : (t + 1) * 512],
                    start=(kc == 0),
                    stop=(kc == NKC - 2),
                    perf_mode=DR,
                )
            nc.vector.tensor_copy(out=dst[:, pr, t * 512 : (t + 1) * 512], in_=ps[:])

        def proj_v(ic):
            """fp8 DoubleRow V projection for one i-chunk via two borrowed
            scores-pool slots (one per 8-head half)."""
            for nb in range(2):
                ps = psS.tile([128, 512], F32, tag="sc", name=f"pjv{ic}_{nb}")
                for kc in range(0, NKC, 2):
                    mm(
                        ps[:],
                        tokT_sb[:, kc : kc + 2, ic * 128 : (ic + 1) * 128],
                        wv_sb[:, kc : kc + 2, nb * 512 : (nb + 1) * 512],
                        start=(kc == 0),
                        stop=(kc == NKC - 2),
                        perf_mode=DR,
                    )
                nc.vector.tensor_copy(
                    out=v_sb[:, ic, nb * 8 : (nb + 1) * 8, 0:DV],
                    in_=ps.rearrange("p (h v) -> p h v", h=8),
                )

        def normalize(pr, acc):
            """multi^T[h] = heads^T / rowsum; runs on DVE/DMA only."""
            for hh in range(2):
                h = 2 * pr + hh
                hraw = pn_pool.tile([DV + 1, JW], F32, tag="hraw", name=f"hraw{h}")
                nc.vector.tensor_copy(out=hraw[:], in_=acc[hh][:])  # frees acc
                rs_dram = pdram.tile([1, JW], F32, tag="rsd", name=f"rsd{h}")
                nc.sync.dma_start(out=rs_dram[:], in_=hraw[DV : DV + 1, :])
                rec_in = pn_pool.tile([DV, JW], F32, tag="rin", name=f"rin{h}")
                nc.gpsimd.dma_start(out=rec_in[:], in_=rs_dram.to_broadcast((DV, JW)))
                nc.vector.reciprocal_approx_fast(out=rec_in[:], in_=rec_in[:])
                if hh == 0:
                    nc.vector.tensor_tensor(
                        multiT[pr // 2][0:64, pr % 2, :], hraw[0:DV, :], rec_in[:], mult
                    )
                else:
                    tmp64 = pn_pool.tile([DV, JW], F8, tag="tmp64", name=f"tmp{h}")
                    nc.vector.tensor_tensor(tmp64[:], hraw[0:DV, :], rec_in[:], mult)
                    nc.sync.dma_start(
                        out=multiT[pr // 2][64:128, pr % 2, :], in_=tmp64[:]
                    )

        def attention(hooks_by_pr, after_pair=None):
            """All pairs, flat: attnV lags one i-chunk behind scores/exp and
            crosses pair boundaries so the exp stream never waits on the PE.
            Hooks and the lag-2 attnV are emitted BEFORE this iteration's
            scores: the scores matmul must wait for an exp to free its PSUM
            slot (ring depth 2), and everything emitted ahead of it in the PE
            chain fills that wait. The attnV lags TWO i-chunks so its eT
            input (the exp output) is long since complete when the PE reaches
            it. hooks_by_pr[pr][ic] is a list of thunks."""
            from collections import deque

            pending = deque()   # (eTs, ic, pr, acc), newest at right
            acc_by_pr = {}

            def do_attnv(peT, pic, ppr, pacc):
                for hh in range(2):
                    for jb in range(2):
                        mm(
                            pacc[hh][:, jb * 512 : (jb + 1) * 512],
                            v_sb[:, pic, 2 * ppr + hh, :],
                            peT[2 * hh + jb][:],
                            start=(pic == 0),
                            stop=(pic == NIC - 1),
                        )

            for pr in range(NPAIR):
                acc_by_pr[pr] = [
                    psAcc.tile([DV + 1, JW], F32, tag="acc", name=f"acc{pr}_{hh}")
                    for hh in range(2)
                ]
                hooks = hooks_by_pr.get(pr, {})
                for ic in range(NIC):
                    for fn in hooks.get(ic, ()):
                        fn()
                    while len(pending) >= 2:
                        pa = pending.popleft()
                        do_attnv(*pa)
                        if pa[1] == NIC - 1:
                            normalize(pa[2], pa[3])
                    # scores^T, row-tiled pair (K=64 at partitions 0/64); one
                    # 1-bank PSUM slot and one exp per (hh, jb) quarter
                    eTs = []
                    for hh in range(2):
                        for jb in range(2):
                            ps_s = psS.tile(
                                [128, 512], F32, tag="sc",
                                name=f"ps_s{pr}_{ic}_{hh}{jb}",
                            )
                            mm(
                                ps_s[:],
                                qT_sb[hh * 64 : (hh + 1) * 64, pr, ic * 128 : (ic + 1) * 128],
                                kT_sb[hh * 64 : (hh + 1) * 64, pr, jb * 512 : (jb + 1) * 512],
                                start=True,
                                stop=True,
                            )
                            eT = pe_pool.tile(
                                [128, 512], BF, tag="eT", name=f"eT{pr}_{ic}_{hh}{jb}"
                            )
                            # x256-scaled q and k: fold 2^-16 into the exp scale
                            nc.scalar.activation(
                                eT[:], ps_s[:], Exp, scale=0.125 / (WSCALE * WSCALE)
                            )
                            eTs.append(eT)
                    pending.append((eTs, ic, pr, acc_by_pr[pr]))
                if after_pair and pr in after_pair:
                    after_pair[pr]()
            while pending:
                pa = pending.popleft()
                do_attnv(*pa)
                if pa[1] == NIC - 1:
                    normalize(pa[2], pa[3])

        # Hook schedule: V projections ride pair 0; each pair p computes its
        # own q chains t1-3 mid-pair and pair p+1's q t0 / k chains late, so
        # every pair's inputs are ready one pair ahead. Pair 7's t1-3 move
        # into pair 6 so tokT dies at pair 6's end (phase C reuses the space).
        hooks = {pr: {} for pr in range(NPAIR)}

        def add_hook(pr, ic, fn):
            hooks[pr].setdefault(ic, []).append(fn)

        for ic in range(2, NIC):
            add_hook(0, ic, lambda ic=ic: proj_v(ic))
        add_hook(0, 1, lambda: proj_v(0))
        add_hook(0, 1, lambda: proj_v(1))
        for pr in range(NPAIR):
            own = pr if pr < 7 else 6
            for t, ic in (
                ((1, 3), (2, 7), (3, 11)) if pr < 7 else ((1, 9), (2, 10), (3, 12))
            ):
                add_hook(own, ic, lambda pr=pr, t=t: proj_chain(pr, "q", t))
            if pr < 7:
                add_hook(pr, 13, lambda pr=pr: proj_chain(pr + 1, "q", 0))
                add_hook(pr, 14, lambda pr=pr: proj_chain(pr + 1, "k", 0))
                add_hook(pr, 15, lambda pr=pr: proj_chain(pr + 1, "k", 1))

        pc_tiles = {}

        def open_phase_c():
            # pa's tensors are dead after pair 6 (pair 7's chains were hoisted
            # into pair 6); reuse the space for phase C inputs so their DMA
            # overlaps pair 7.
            pa.release()
            pc = stack.enter_context(tc.tile_pool(name="pc", bufs=1))
            pc_tiles["wo"] = pc.tile([128, NKC, D], F8, name="wo_sb")
            pc_tiles["tokres"] = pc.tile([128, NJCH, D], F32, name="tokres_sb")
            nc.sync.dma_start(pc_tiles["wo"][:], wo_d[:])
            nc.sync.dma_start(pc_tiles["tokres"][:], tokres_d[:])

        # upfront: just enough projection for pair 0's first scores
        proj_chain(0, "q", 0)
        proj_chain(0, "k", 0)
        proj_chain(0, "k", 1)

        attention(hooks, after_pair={6: open_phase_c})
        wo_sb = pc_tiles["wo"]
        tokres_sb = pc_tiles["tokres"]
        psAcc.release()
        psS.release()
        # ---------------- Phase C: out-proj + residual + LayerNorm ----------------
        with (
            tc.tile_pool(name="pC", bufs=2) as pC,
            tc.tile_pool(name="pStats", bufs=8) as pStats,
            tc.tile_pool(name="psC", bufs=4, space="PSUM") as psC,
        ):
            # Out-proj in two steps per jch: kc 0-6 accumulate early (their
            # multiT chunks are ready pairs before the last normalize), kc 7
            # finishes when multiT[7] lands. Prefilling 4 PSUM groups hides
            # the last normalize's DRAM round-trip behind ~12us of matmuls.
            prefill = {}

            def emit_prefill(jch):
                ps_o = psC.tile([128, D], F32, tag="po", name=f"ps_o{jch}")
                for kcp in range(NKC // 2 - 1):
                    lhsT = multiT[kcp][:, :, jch * 128 : (jch + 1) * 128]
                    for nb in range(2):
                        mm(
                            ps_o[:, nb * 512 : (nb + 1) * 512],
                            lhsT,
                            wo_sb[:, 2 * kcp : 2 * kcp + 2, nb * 512 : (nb + 1) * 512],
                            start=(kcp == 0),
                            stop=False,
                            perf_mode=DR,
                        )
                prefill[jch] = ps_o

            for jch in range(4):
                emit_prefill(jch)
            for jch in range(NJCH):
                ps_o = prefill.pop(jch)
                lhsT = multiT[NKC // 2 - 1][:, :, jch * 128 : (jch + 1) * 128]
                for nb in range(2):
                    mm(
                        ps_o[:, nb * 512 : (nb + 1) * 512],
                        lhsT,
                        wo_sb[:, NKC - 2 : NKC, nb * 512 : (nb + 1) * 512],
                        start=False,
                        stop=True,
                        perf_mode=DR,
                    )
                # x = psum + residual, sum_t = rowsum(x), in one DVE pass
                x_sb = pC.tile([128, D], F32, tag="x", name=f"x{jch}")
                sum_t = pStats.tile([128, 1], F32, tag="sum", name=f"sum{jch}")
                # x = psum/WSCALE + residual (undo the Wo fp8 pre-scale),
                # sum_t = rowsum(x), in one DVE pass
                nc.vector.scalar_tensor_tensor(
                    out=x_sb[:],
                    in0=ps_o[:],
                    scalar=1.0 / WSCALE,
                    in1=tokres_sb[:, jch, :],
                    op0=mult,
                    op1=add,
                    accum_out=sum_t[:],
                )
                negmean = pStats.tile([128, 1], F32, tag="nm", name=f"nm{jch}")
                nc.vector.tensor_scalar_mul(negmean[:], sum_t[:], -1.0 / D)
                # ssq = sum((x-m)^2) on the otherwise-idle Scalar engine:
                # Square(x*1 + negmean) with accum_out
                scrap = pC.tile([128, D], BF, tag="scrap", name=f"scrap{jch}")
                ssq = pStats.tile([128, 1], F32, tag="ssq", name=f"ssq{jch}")
                nc.scalar.activation(
                    scrap[:], x_sb[:], Square, bias=negmean[:], accum_out=ssq[:]
                )
                std_t = pStats.tile([128, 1], F32, tag="std", name=f"std{jch}")
                nc.scalar.activation(std_t[:], ssq[:], Sqrt, bias=eps_sb[:], scale=1.0 / D)
                rstd = pStats.tile([128, 1], F32, tag="rstd", name=f"rstd{jch}")
                nc.vector.reciprocal(rstd[:], std_t[:])
                # (x - m) * rstd == x*rstd + (negmean*rstd), one ACT op
                rstd_nm = pStats.tile([128, 1], F32, tag="rnm", name=f"rnm{jch}")
                nc.vector.tensor_tensor(rstd_nm[:], negmean[:], rstd[:], mult)
                out_sb = pC.tile([128, D], F32, tag="out", name=f"out{jch}")
                nc.scalar.activation(
                    out_sb[:],
                    x_sb[:],
                    mybir.ActivationFunctionType.Identity,
                    bias=rstd_nm[:],
                    scale=rstd[:],
                )
                if apply_affine:
                    nc.gpsimd.tensor_tensor(out_sb[:], out_sb[:], gamma_sb[:], mult)
                    nc.gpsimd.tensor_tensor(out_sb[:], out_sb[:], beta_sb[:], add)
                nc.sync.dma_start(out_d[:, jch], out_sb[:])
                # second prefill wave once the first four STTs are emitted, so
                # the PE chain never parks on a not-yet-freed PSUM group
                if jch == 3:
                    for j2 in range(4, NJCH):
                        emit_prefill(j2)

    nc.compile()
    return nc


def _prep_inputs(tokens, Wq, Wk, Wv, Wo, gamma, beta):
    """Host-side layout prep. Returns per-core input maps."""
    tokens = np.ascontiguousarray(np.asarray(tokens, dtype=np.float32))
    # weights -> [p, kc, n] with row index kc*128+p
    def rows128(a):  # [1024, N] -> [128, 8, N]
        return np.ascontiguousarray(
            a.reshape(NKC, 128, a.shape[-1]).transpose(1, 0, 2)
        )

    wq_all = rows128(
        (np.asarray(Wq).transpose(1, 0, 2).reshape(D, H * DK) * WSCALE).astype(FP8)
    )
    wk_all = rows128(
        (np.asarray(Wk).transpose(1, 0, 2).reshape(D, H * DK) * WSCALE).astype(FP8)
    )
    wv_all = rows128(
        (np.asarray(Wv).transpose(1, 0, 2).reshape(D, H * DV) * WSCALE).astype(FP8)
    )
    wo_all = rows128((np.asarray(Wo) * WSCALE).astype(FP8))
    gamma_bc = np.ascontiguousarray(
        np.broadcast_to(np.asarray(gamma, np.float32), (128, D))
    )
    beta_bc = np.ascontiguousarray(
        np.broadcast_to(np.asarray(beta, np.float32), (128, D))
    )

    tokT_by_b = []
    for b in range(B):
        tokT_by_b.append(rows128(tokens[b].T.astype(FP8)))  # [128, 8, 2048]

    in_maps = []
    for c in range(NCORES):
        b, jc = c // 2, c % 2
        tokT = tokT_by_b[b]
        tokTj = np.ascontiguousarray(tokT[:, :, jc * JW : (jc + 1) * JW])
        tokres = np.ascontiguousarray(
            tokens[b, jc * JW : (jc + 1) * JW]
            .reshape(NJCH, 128, D)
            .transpose(1, 0, 2)
        )
        in_maps.append(
            {
                "tokT": tokT,
                "tokTj": tokTj,
                "wq": wq_all,
                "wk": wk_all,
                "wv": wv_all,
                "wo": wo_all,
                "tokres": tokres,
                "gamma_bc": gamma_bc,
                "beta_bc": beta_bc,
            }
        )
    return in_maps


def run(inputs, trace=False, tmpdir=None):
    """Run on hardware; returns (output, BassKernelResults)."""
    from concourse.bass_utils import run_bass_kernel_spmd

    apply_affine = not (
        np.all(np.asarray(inputs["gamma"]) == 1.0)
        and np.all(np.asarray(inputs["beta"]) == 0.0)
    )
    key = ("nc", apply_affine)
    if key not in _CACHE:
        _CACHE[key] = _build_nc(apply_affine)
    nc = _CACHE[key]
    in_maps = _prep_inputs(**inputs)
    res = run_bass_kernel_spmd(
        nc, in_maps, core_ids=list(range(NCORES)), trace=trace, tmpdir=tmpdir
    )
    out = np.empty((B, S, D), np.float32)
    for c in range(NCORES):
        b, jc = c // 2, c % 2
        o = res.results[c]["out"]  # [128, 8, 1024]
        out[b, jc * JW : (jc + 1) * JW] = (
            o.transpose(1, 0, 2).reshape(JW, D)
        )
    return out, res


def kernel(tokens, Wq, Wk, Wv, Wo, gamma, beta):
    out, _ = run(
        dict(tokens=tokens, Wq=Wq, Wk=Wk, Wv=Wv, Wo=Wo, gamma=gamma, beta=beta)
    )
    return out


# revision 30
# speedup vs baseline: 1.3766x; 1.0079x over previous
"""TRN2 Bass kernel for fused MHA (softmax-over-query quirk) + out-proj + residual + LayerNorm.

Problem shapes (hardcoded): tokens [4,2048,1024], Wq/Wk [16,1024,64], Wv [16,1024,64],
Wo [1024,1024], gamma/beta [1024]. Output [4,2048,1024] fp32.

Sharding: 8 cores, core c owns (batch b=c//2, S-half jc=c%2) of the OUTPUT rows.
No collectives. Each core computes, for its batch b:
  qT[dk,i] (full S), kT[dk,j] (its half), V[i,dv] (full S) in bf16,
  scores^T[i,j] = q_i.k_j (PSUM fp32), e = exp(scores/8) (bf16),
  heads^T[dv,j] + rowsum row via a ones-column appended to V,
  multi^T = heads^T / rowsum, out = multi @ Wo + tokens, LayerNorm rows.

QKV projections run in fp8e4m3 with DoubleRow perf mode (2 K-planes per
matmul): tokens cast to fp8, weights scaled x256 (dodges e4m3 subnormals)
and cast to fp8. The x256 scale cancels: scores pick up 2^16 (folded into
the exp scale) and heads/rowsum both pick up 2^8 (ones column = 256).
Projections are interleaved into the attention stream as hooks so the
Scalar-engine exp stream starts ~10us in instead of after all projections.
Attention math in bf16 matmuls with fp32 PSUM; residual + LN in fp32.
"""

import numpy as np
import ml_dtypes

BF16 = ml_dtypes.bfloat16
FP8 = ml_dtypes.float8_e4m3

B, S, D, H, DK, DV = 4, 2048, 1024, 16, 64, 64
NCORES = 8
NPAIR = 8     # head pairs
NKC = 8       # D // 128 contraction chunks
NIC = 16      # S // 128 i-chunks
JW = 1024     # j columns per core (S/2)
NJCH = 8      # JW // 128
LN_EPS = 1e-5
WSCALE = 256.0  # fp8 weight pre-scale (power of 2)

_CACHE = {}


def _build_nc(apply_affine):
    import concourse.tile as tile
    from concourse import bacc, mybir

    F32 = mybir.dt.float32
    BF = mybir.dt.bfloat16
    F8 = mybir.dt.float8e4
    Exp = mybir.ActivationFunctionType.Exp
    Square = mybir.ActivationFunctionType.Square
    Sqrt = mybir.ActivationFunctionType.Sqrt
    mult = mybir.AluOpType.mult
    add = mybir.AluOpType.add
    DR = mybir.MatmulPerfMode.DoubleRow

    nc = bacc.Bacc(
        "TRN2",
        target_bir_lowering=False,
        debug=False,
        enable_asserts=False,
        num_devices=NCORES,
    )

    # DRAM I/O (per-core views; host prepares layouts)
    tokT_d = nc.dram_tensor("tokT", (128, NKC, S), F8, kind="ExternalInput").ap()
    tokTj_d = nc.dram_tensor("tokTj", (128, NKC, JW), F8, kind="ExternalInput").ap()
    wq_d = nc.dram_tensor("wq", (128, NKC, H * DK), F8, kind="ExternalInput").ap()
    wk_d = nc.dram_tensor("wk", (128, NKC, H * DK), F8, kind="ExternalInput").ap()
    wv_d = nc.dram_tensor("wv", (128, NKC, H * DV), F8, kind="ExternalInput").ap()
    wo_d = nc.dram_tensor("wo", (128, NKC, D), F8, kind="ExternalInput").ap()
    tokres_d = nc.dram_tensor("tokres", (128, NJCH, D), F32, kind="ExternalInput").ap()
    if apply_affine:
        gamma_d = nc.dram_tensor("gamma_bc", (128, D), F32, kind="ExternalInput").ap()
        beta_d = nc.dram_tensor("beta_bc", (128, D), F32, kind="ExternalInput").ap()
    out_d = nc.dram_tensor("out", (128, NJCH, D), F32, kind="ExternalOutput").ap()
    from contextlib import ExitStack

    from concourse.bass import _add_dep_helper

    # Chain all PE matmuls in emission order: stops the scheduler from
    # interleaving row-conflicting matmuls and keeps the stream dense.
    _prev_mm = [None]

    def mm(*args, **kwargs):
        inst = nc.tensor.matmul(*args, **kwargs)
        if _prev_mm[0] is not None:
            _add_dep_helper(inst.ins, _prev_mm[0].ins, sync=False, reason="pe-order")
        _prev_mm[0] = inst
        return inst

    with tile.TileContext(nc) as tc, ExitStack() as stack:
        persist = stack.enter_context(tc.tile_pool(name="persist", bufs=1))
        qT_sb = persist.tile([128, NPAIR, S], BF)          # [pair-dk, pr, i]
        kT_sb = persist.tile([128, NPAIR, JW], BF)         # [pair-dk, pr, j]
        v_sb = persist.tile([128, NIC, H, DV + 1], BF)     # [i%128, ic, h, dv|256s]
        # multi^T in fp8, one tile per KC-PAIR ([128, 2, JW], middle dim = the
        # DoubleRow K-plane) so the out-proj runs fp8 DoubleRow; per-pair-ish
        # tiles keep out-proj dep tracking from serializing on the last write
        multiT = [
            persist.tile([128, 2, JW], F8, name=f"multiT{i}") for i in range(NKC // 2)
        ]
        eps_sb = persist.tile([128, 1], F32)
        ones1_sb = persist.tile([1, DV], F32)
        nc.vector.memset(ones1_sb[:], 1.0)
        if apply_affine:
            gamma_sb = persist.tile([128, D], F32)
            beta_sb = persist.tile([128, D], F32)
            nc.sync.dma_start(gamma_sb[:], gamma_d[:])
            nc.sync.dma_start(beta_sb[:], beta_d[:])
        nc.vector.memset(eps_sb[:], LN_EPS)
        for ic in range(NIC):
            # ones column scaled by WSCALE so rowsum matches the x256 V scale
            nc.vector.memset(v_sb[:, ic, :, DV : DV + 1], WSCALE)

        # pools that outlive pa must be allocated first (LIFO release)
        # psS: four 1-bank [128,512] slots. Depth 4 (vs 2x [128,1024]) means a
        # scores matmul waits on an exp from ~2 i-chunks back, so projection
        # hooks borrowing a slot no longer stall the exp stream.
        psS = tc.alloc_tile_pool(name="psS", bufs=4, space="PSUM")
        psAcc = tc.alloc_tile_pool(name="psAcc", bufs=2, space="PSUM")
        pe_pool = stack.enter_context(tc.tile_pool(name="pe", bufs=16))
        pn_pool = stack.enter_context(tc.tile_pool(name="pn", bufs=2))
        pdram = stack.enter_context(tc.tile_pool(name="pdram", bufs=2, space="DRAM"))

        pa = tc.alloc_tile_pool(name="pa", bufs=1)
        wq_sb = pa.tile([128, NKC, H * DK], F8)
        wk_sb = pa.tile([128, NKC, H * DK], F8)
        tokT_sb = pa.tile([128, NKC, S], F8)
        tokTj_sb = pa.tile([128, NKC, JW], F8)
        wv_sb = pa.tile([128, NKC, H * DV], F8)

        # Startup DMA over both HWDGE rings (SP + ACT, independent FIFOs).
        # Critical path to the first exp is only the pair-0 weight slices +
        # tokens-i0 + tokTj; everything else queues behind.
        nc.sync.dma_start(wq_sb[:, :, 0:128], wq_d[:, :, 0:128])
        nc.sync.dma_start(wk_sb[:, :, 0:128], wk_d[:, :, 0:128])
        # tokens i 0..511 first (q chain t0, V chains), as single 3D transfers
        nc.scalar.dma_start(tokT_sb[:, :, 0:512], tokT_d[:, :, 0:512])
        nc.scalar.dma_start(tokTj_sb[:], tokTj_d[:])
        nc.sync.dma_start(tokT_sb[:, :, 512:1024], tokT_d[:, :, 512:1024])
        nc.scalar.dma_start(wv_sb[:], wv_d[:])
        nc.sync.dma_start(tokT_sb[:, :, 1024:2048], tokT_d[:, :, 1024:2048])
        nc.sync.dma_start(wq_sb[:, :, 128:], wq_d[:, :, 128:])
        nc.sync.dma_start(wk_sb[:, :, 128:], wk_d[:, :, 128:])

        def proj_chain(pr, which, t):
            """One 512-wide fp8 DoubleRow projection chain via a borrowed
            scores-pool slot."""
            w_sb, dst, rhs_sb = (
                (wq_sb, qT_sb, tokT_sb) if which == "q" else (wk_sb, kT_sb, tokTj_sb)
            )
            ps = psS.tile([128, 512], F32, tag="sc", name=f"pj{which}{pr}_{t}")
            for kc in range(0, NKC, 2):
                mm(
                    ps[:],
                    w_sb[:, kc : kc + 2, pr * 128 : (pr + 1) * 128],
                    rhs_sb[:, kc : kc + 2, t * 512 : (t + 1) * 512],
                    start=(kc == 0),
                    stop=(kc == NKC - 2),
                    perf_mode=DR,
                )
            nc.vector.tensor_copy(out=dst[:, pr, t * 512 : (t + 1) * 512], in_=ps[:])

        def proj_v(ic):
            """fp8 DoubleRow V projection for one i-chunk via two borrowed
            scores-pool slots (one per 8-head half)."""
            for nb in range(2):
                ps = psS.tile([128, 512], F32, tag="sc", name=f"pjv{ic}_{nb}")
                for kc in range(0, NKC, 2):
                    mm(
                        ps[:],
                        tokT_sb[:, kc : kc + 2, ic * 128 : (ic + 1) * 128],
                        wv_sb[:, kc : kc + 2, nb * 512 : (nb + 1) * 512],
                        start=(kc == 0),
                        stop=(kc == NKC - 2),
                        perf_mode=DR,
                    )
                nc.vector.tensor_copy(
                    out=v_sb[:, ic, nb * 8 : (nb + 1) * 8, 0:DV],
                    in_=ps.rearrange("p (h v) -> p h v", h=8),
                )

        def normalize(pr, acc):
            """multi^T[h] = heads^T / rowsum; runs on DVE/DMA only."""
            for hh in range(2):
                h = 2 * pr + hh
                hraw = pn_pool.tile([DV + 1, JW], F32, tag="hraw", name=f"hraw{h}")
                nc.vector.tensor_copy(out=hraw[:], in_=acc[hh][:])  # frees acc
                rs_dram = pdram.tile([1, JW], F32, tag="rsd", name=f"rsd{h}")
                nc.sync.dma_start(out=rs_dram[:], in_=hraw[DV : DV + 1, :])
                rec_in = pn_pool.tile([DV, JW], F32, tag="rin", name=f"rin{h}")
                nc.gpsimd.dma_start(out=rec_in[:], in_=rs_dram.to_broadcast((DV, JW)))
                nc.vector.reciprocal_approx_fast(out=rec_in[:], in_=rec_in[:])
                if hh == 0:
                    nc.vector.tensor_tensor(
                        multiT[pr // 2][0:64, pr % 2, :], hraw[0:DV, :], rec_in[:], mult
                    )
                else:
                    tmp64 = pn_pool.tile([DV, JW], F8, tag="tmp64", name=f"tmp{h}")
                    nc.vector.tensor_tensor(tmp64[:], hraw[0:DV, :], rec_in[:], mult)
                    nc.sync.dma_start(
                        out=multiT[pr // 2][64:128, pr % 2, :], in_=tmp64[:]
                    )

        def attention(hooks_by_pr, after_pair=None):
            """All pairs, flat: attnV lags one i-chunk behind scores/exp and
            crosses pair boundaries so the exp stream never waits on the PE.
            Hooks and the lag-2 attnV are emitted BEFORE this iteration's
            scores: the scores matmul must wait for an exp to free its PSUM
            slot (ring depth 2), and everything emitted ahead of it in the PE
            chain fills that wait. The attnV lags TWO i-chunks so its eT
            input (the exp output) is long since complete when the PE reaches
            it. hooks_by_pr[pr][ic] is a list of thunks."""
            from collections import deque

            pending = deque()   # (eTs, ic, pr, acc), newest at right
            acc_by_pr = {}

            def do_attnv(peT, pic, ppr, pacc):
                for hh in range(2):
                    for jb in range(2):
                        mm(
                            pacc[hh][:, jb * 512 : (jb + 1) * 512],
                            v_sb[:, pic, 2 * ppr + hh, :],
                            peT[2 * hh + jb][:],
                            start=(pic == 0),
                            stop=(pic == NIC - 1),
                        )

            for pr in range(NPAIR):
                acc_by_pr[pr] = [
                    psAcc.tile([DV + 1, JW], F32, tag="acc", name=f"acc{pr}_{hh}")
                    for hh in range(2)
                ]
                hooks = hooks_by_pr.get(pr, {})
                for ic in range(NIC):
                    for fn in hooks.get(ic, ()):
                        fn()
                    while len(pending) >= 3:
                        pa = pending.popleft()
                        do_attnv(*pa)
                        if pa[1] == NIC - 1:
                            normalize(pa[2], pa[3])
                    # scores^T, row-tiled pair (K=64 at partitions 0/64); one
                    # 1-bank PSUM slot and one exp per (hh, jb) quarter
                    eTs = []
                    for hh in range(2):
                        for jb in range(2):
                            ps_s = psS.tile(
                                [128, 512], F32, tag="sc",
                                name=f"ps_s{pr}_{ic}_{hh}{jb}",
                            )
                            mm(
                                ps_s[:],
                                qT_sb[hh * 64 : (hh + 1) * 64, pr, ic * 128 : (ic + 1) * 128],
                                kT_sb[hh * 64 : (hh + 1) * 64, pr, jb * 512 : (jb + 1) * 512],
                                start=True,
                                stop=True,
                            )
                            eT = pe_pool.tile(
                                [128, 512], BF, tag="eT", name=f"eT{pr}_{ic}_{hh}{jb}"
                            )
                            # x256-scaled q and k: fold 2^-16 into the exp scale
                            nc.scalar.activation(
                                eT[:], ps_s[:], Exp, scale=0.125 / (WSCALE * WSCALE)
                            )
                            eTs.append(eT)
                    pending.append((eTs, ic, pr, acc_by_pr[pr]))
                if after_pair and pr in after_pair:
                    after_pair[pr]()
            while pending:
                pa = pending.popleft()
                do_attnv(*pa)
                if pa[1] == NIC - 1:
                    normalize(pa[2], pa[3])

        # Hook schedule: V projections ride pair 0; each pair p computes its
        # own q chains t1-3 mid-pair and pair p+1's q t0 / k chains late, so
        # every pair's inputs are ready one pair ahead. Pair 7's t1-3 move
        # into pair 6 so tokT dies at pair 6's end (phase C reuses the space).
        hooks = {pr: {} for pr in range(NPAIR)}

        def add_hook(pr, ic, fn):
            hooks[pr].setdefault(ic, []).append(fn)

        for ic in range(2, NIC):
            add_hook(0, ic, lambda ic=ic: proj_v(ic))
        add_hook(0, 1, lambda: proj_v(0))
        add_hook(0, 1, lambda: proj_v(1))
        for pr in range(NPAIR):
            own = pr if pr < 7 else 6
            for t, ic in (
                ((1, 3), (2, 7), (3, 11)) if pr < 7 else ((1, 2), (2, 6), (3, 10))
            ):
                add_hook(own, ic, lambda pr=pr, t=t: proj_chain(pr, "q", t))
            if pr < 7:
                add_hook(pr, 5, lambda pr=pr: proj_chain(pr + 1, "q", 0))
                add_hook(pr, 9, lambda pr=pr: proj_chain(pr + 1, "k", 0))
                add_hook(pr, 13, lambda pr=pr: proj_chain(pr + 1, "k", 1))

        pc_tiles = {}

        def open_phase_c():
            # pa's tensors are dead after pair 6 (pair 7's chains were hoisted
            # into pair 6); reuse the space for phase C inputs so their DMA
            # overlaps pair 7.
            pa.release()
            pc = stack.enter_context(tc.tile_pool(name="pc", bufs=1))
            pc_tiles["wo"] = pc.tile([128, NKC, D], F8, name="wo_sb")
            pc_tiles["tokres"] = pc.tile([128, NJCH, D], F32, name="tokres_sb")
            nc.sync.dma_start(pc_tiles["wo"][:], wo_d[:])
            nc.sync.dma_start(pc_tiles["tokres"][:], tokres_d[:])

        # upfront: just enough projection for pair 0's first scores
        proj_chain(0, "q", 0)
        proj_chain(0, "k", 0)
        proj_chain(0, "k", 1)

        attention(hooks, after_pair={6: open_phase_c})
        wo_sb = pc_tiles["wo"]
        tokres_sb = pc_tiles["tokres"]
        psAcc.release()
        psS.release()
        # ---------------- Phase C: out-proj + residual + LayerNorm ----------------
        with (
            tc.tile_pool(name="pC", bufs=4) as pC,
            tc.tile_pool(name="pStats", bufs=8) as pStats,
            tc.tile_pool(name="psC", bufs=4, space="PSUM") as psC,
        ):
            # Out-proj in two steps per jch: kc 0-6 accumulate early (their
            # multiT chunks are ready pairs before the last normalize), kc 7
            # finishes when multiT[7] lands. Prefilling 4 PSUM groups hides
            # the last normalize's DRAM round-trip behind ~12us of matmuls.
            prefill = {}

            def emit_prefill(jch):
                ps_o = psC.tile([128, D], F32, tag="po", name=f"ps_o{jch}")
                for kcp in range(NKC // 2 - 1):
                    lhsT = multiT[kcp][:, :, jch * 128 : (jch + 1) * 128]
                    for nb in range(2):
                        mm(
                            ps_o[:, nb * 512 : (nb + 1) * 512],
                            lhsT,
                            wo_sb[:, 2 * kcp : 2 * kcp + 2, nb * 512 : (nb + 1) * 512],
                            start=(kcp == 0),
                            stop=False,
                            perf_mode=DR,
                        )
                prefill[jch] = ps_o

            for jch in range(4):
                emit_prefill(jch)
            for jch in range(NJCH):
                ps_o = prefill.pop(jch)
                lhsT = multiT[NKC // 2 - 1][:, :, jch * 128 : (jch + 1) * 128]
                for nb in range(2):
                    mm(
                        ps_o[:, nb * 512 : (nb + 1) * 512],
                        lhsT,
                        wo_sb[:, NKC - 2 : NKC, nb * 512 : (nb + 1) * 512],
                        start=False,
                        stop=True,
                        perf_mode=DR,
                    )
                # x = psum + residual, sum_t = rowsum(x), in one DVE pass
                x_sb = pC.tile([128, D], F32, tag="x", name=f"x{jch}")
                sum_t = pStats.tile([128, 1], F32, tag="sum", name=f"sum{jch}")
                # x = psum/WSCALE + residual (undo the Wo fp8 pre-scale),
                # sum_t = rowsum(x), in one DVE pass
                nc.vector.scalar_tensor_tensor(
                    out=x_sb[:],
                    in0=ps_o[:],
                    scalar=1.0 / WSCALE,
                    in1=tokres_sb[:, jch, :],
                    op0=mult,
                    op1=add,
                    accum_out=sum_t[:],
                )
                negmean = pStats.tile([128, 1], F32, tag="nm", name=f"nm{jch}")
                nc.vector.tensor_scalar_mul(negmean[:], sum_t[:], -1.0 / D)
                # ssq = sum((x-m)^2) on the otherwise-idle Scalar engine:
                # Square(x*1 + negmean) with accum_out
                scrap = pC.tile([128, D], BF, tag="scrap", name=f"scrap{jch}")
                ssq = pStats.tile([128, 1], F32, tag="ssq", name=f"ssq{jch}")
                nc.scalar.activation(
                    scrap[:], x_sb[:], Square, bias=negmean[:], accum_out=ssq[:]
                )
                std_t = pStats.tile([128, 1], F32, tag="std", name=f"std{jch}")
                nc.scalar.activation(std_t[:], ssq[:], Sqrt, bias=eps_sb[:], scale=1.0 / D)
                rstd = pStats.tile([128, 1], F32, tag="rstd", name=f"rstd{jch}")
                nc.vector.reciprocal(rstd[:], std_t[:])
                # (x - m) * rstd == x*rstd + (negmean*rstd), one ACT op
                rstd_nm = pStats.tile([128, 1], F32, tag="rnm", name=f"rnm{jch}")
                nc.vector.tensor_tensor(rstd_nm[:], negmean[:], rstd[:], mult)
                out_sb = pC.tile([128, D], F32, tag="out", name=f"out{jch}")
                nc.scalar.activation(
                    out_sb[:],
                    x_sb[:],
                    mybir.ActivationFunctionType.Identity,
                    bias=rstd_nm[:],
                    scale=rstd[:],
                )
                if apply_affine:
                    nc.gpsimd.tensor_tensor(out_sb[:], out_sb[:], gamma_sb[:], mult)
                    nc.gpsimd.tensor_tensor(out_sb[:], out_sb[:], beta_sb[:], add)
                nc.sync.dma_start(out_d[:, jch], out_sb[:])
                # second prefill wave once the first four STTs are emitted, so
                # the PE chain never parks on a not-yet-freed PSUM group
                if jch == 3:
                    for j2 in range(4, NJCH):
                        emit_prefill(j2)

    nc.compile()
    return nc


def _prep_inputs(tokens, Wq, Wk, Wv, Wo, gamma, beta):
    """Host-side layout prep. Returns per-core input maps."""
    tokens = np.ascontiguousarray(np.asarray(tokens, dtype=np.float32))
    # weights -> [p, kc, n] with row index kc*128+p
    def rows128(a):  # [1024, N] -> [128, 8, N]
        return np.ascontiguousarray(
            a.reshape(NKC, 128, a.shape[-1]).transpose(1, 0, 2)
        )

    wq_all = rows128(
        (np.asarray(Wq).transpose(1, 0, 2).reshape(D, H * DK) * WSCALE).astype(FP8)
    )
    wk_all = rows128(
        (np.asarray(Wk).transpose(1, 0, 2).reshape(D, H * DK) * WSCALE).astype(FP8)
    )
    wv_all = rows128(
        (np.asarray(Wv).transpose(1, 0, 2).reshape(D, H * DV) * WSCALE).astype(FP8)
    )
    wo_all = rows128((np.asarray(Wo) * WSCALE).astype(FP8))
    gamma_bc = np.ascontiguousarray(
        np.broadcast_to(np.asarray(gamma, np.float32), (128, D))
    )
    beta_bc = np.ascontiguousarray(
        np.broadcast_to(np.asarray(beta, np.float32), (128, D))
    )

    tokT_by_b = []
    for b in range(B):
        tokT_by_b.append(rows128(tokens[b].T.astype(FP8)))  # [128, 8, 2048]

    in_maps = []
    for c in range(NCORES):
        b, jc = c // 2, c % 2
        tokT = tokT_by_b[b]
        tokTj = np.ascontiguousarray(tokT[:, :, jc * JW : (jc + 1) * JW])
        tokres = np.ascontiguousarray(
            tokens[b, jc * JW : (jc + 1) * JW]
            .reshape(NJCH, 128, D)
            .transpose(1, 0, 2)
        )
        in_maps.append(
            {
                "tokT": tokT,
                "tokTj": tokTj,
                "wq": wq_all,
                "wk": wk_all,
                "wv": wv_all,
                "wo": wo_all,
                "tokres": tokres,
                "gamma_bc": gamma_bc,
                "beta_bc": beta_bc,
            }
        )
    return in_maps


def run(inputs, trace=False, tmpdir=None):
    """Run on hardware; returns (output, BassKernelResults)."""
    from concourse.bass_utils import run_bass_kernel_spmd

    apply_affine = not (
        np.all(np.asarray(inputs["gamma"]) == 1.0)
        and np.all(np.asarray(inputs["beta"]) == 0.0)
    )
    key = ("nc", apply_affine)
    if key not in _CACHE:
        _CACHE[key] = _build_nc(apply_affine)
    nc = _CACHE[key]
    in_maps = _prep_inputs(**inputs)
    res = run_bass_kernel_spmd(
        nc, in_maps, core_ids=list(range(NCORES)), trace=trace, tmpdir=tmpdir
    )
    out = np.empty((B, S, D), np.float32)
    for c in range(NCORES):
        b, jc = c // 2, c % 2
        o = res.results[c]["out"]  # [128, 8, 1024]
        out[b, jc * JW : (jc + 1) * JW] = (
            o.transpose(1, 0, 2).reshape(JW, D)
        )
    return out, res


def kernel(tokens, Wq, Wk, Wv, Wo, gamma, beta):
    out, _ = run(
        dict(tokens=tokens, Wq=Wq, Wk=Wk, Wv=Wv, Wo=Wo, gamma=gamma, beta=beta)
    )
    return out
